# revision 1
# baseline (speedup 1.0000x reference)
"""Trainium2 Bass kernel for AdvancedGATModel (4-layer edge-featured GAT +
Set2Set pooling + MLP head), sharded across 8 NeuronCores.

Sharding: nodes are split into 8 contiguous slices (6250 each); each core owns
the edges whose *destination* lands in its slice (plus self-loops), so segment
softmax and the scatter-add aggregation are core-local.  Per layer each core
computes the linear transform of its node slice, all cores AllGather the
transformed features into a replicated table, and each core gathers its edges'
source rows via indirect DMA.  Segment softmax/weighted-sum run as dense
128-edge-tile matmuls against on-the-fly one-hot "indicator" matrices
(edges x window-nodes).  Set2Set is sharded by graph (64 graphs/core; graphs
never straddle cores), and the [64,5] head outputs are concatenated on host.

The program is identical on all 8 cores (SPMD); only input *data* differs.
All shapes below are hardcoded for the grading problem.
"""

import numpy as np

import concourse.bass as bass
import concourse.bacc as bacc
import concourse.tile as tile
import concourse.mybir as mybir
from concourse.bass_utils import run_bass_kernel_spmd

F32 = mybir.dt.float32
I32 = mybir.dt.int32
AF = mybir.ActivationFunctionType
OP = mybir.AluOpType
P = 128


class CFG:
    # full problem; small-mode tests override these
    N = 50000          # nodes
    E = 800000         # edges (before self loops)
    G = 512            # graphs
    ND = 14            # node feat dim
    ED = 4             # edge feat dim
    GD = 13            # global feat dim
    D = 256            # hidden
    H = 8              # heads
    C = 32             # per-head channels
    NC = 8             # cores
    NPC = N // NC      # nodes per core = 6250
    W = 49             # node windows per core (ceil(NPC/128))
    K = 20             # edge tiles per window (capacity K*128 edges)
    GPC = G // NC      # graphs per core = 64
    TS = 51            # set2set node tiles per core (capacity TS*128 nodes)
    S2S_STEPS = 3
    LAYERS = 4
    FAKE_AG = False     # replace AllGathers with local copies (debug only)

    @classmethod
    def derive(cls):
        cls.NPC = cls.N // cls.NC
        cls.GPC = cls.G // cls.NC
        cls.W = -(-cls.NPC // P)
        return cls


def make_small_cfg():
    class Small(CFG):
        N = 2048
        E = 8192
        G = 64
        W = 2
        K = 8
        TS = 3
    return Small.derive()


# ------------------------------------------------------------------
# host-side preprocessing
# ------------------------------------------------------------------

def host_prep(inp, cfg):
    """Build per-core input maps from the full input dict."""
    N, E, G = cfg.N, cfg.E, cfg.G
    NC, NPC, W, K, GPC, TS = cfg.NC, cfg.NPC, cfg.W, cfg.K, cfg.GPC, cfg.TS
    D, H, C, ED = cfg.D, cfg.H, cfg.C, cfg.ED

    src = np.asarray(inp["edge_index"][0])
    dst = np.asarray(inp["edge_index"][1])
    ea = np.asarray(inp["edge_attr"], dtype=np.float32)
    batch = np.asarray(inp["batch_idx"])
    x = np.asarray(inp["x"], dtype=np.float32)

    # self-loop attr = mean incoming edge attr (0 for isolated nodes)
    deg = np.bincount(dst, minlength=N).astype(np.float32)
    loop = np.zeros((N, ED), np.float32)
    for j in range(ED):
        loop[:, j] = np.bincount(dst, weights=ea[:, j], minlength=N)
    loop /= np.maximum(deg, 1.0)[:, None]

    src2 = np.concatenate([src, np.arange(N, dtype=np.int64)])
    dst2 = np.concatenate([dst, np.arange(N, dtype=np.int64)])
    ea2 = np.concatenate([ea, loop], axis=0).astype(np.float32)

    order = np.argsort(dst2, kind="stable")
    s_src = src2[order]
    s_dst = dst2[order]
    s_ea = ea2[order]

    # per-layer host-packed weight helpers
    def pack_layer(i):
        Wm = np.asarray(inp[f"g{i}_W"], np.float32)          # [din, H*C]
        We = np.asarray(inp[f"g{i}_We"], np.float32)         # [ED, H*C]
        asrc = np.asarray(inp[f"g{i}_asrc"], np.float32)     # [h, c]
        adst = np.asarray(inp[f"g{i}_adst"], np.float32)
        aedge = np.asarray(inp[f"g{i}_aedge"], np.float32)
        b = np.asarray(inp[f"g{i}_b"], np.float32)
        h, c = asrc.shape
        hc = h * c
        # M_ae[d, h] = sum_c We[d, h*c+cc] * aedge[h, cc]
        M_ae = (We.reshape(ED, h, c) * aedge[None]).sum(-1)  # [ED, h]
        return dict(W=Wm, asrc_flat=asrc.reshape(1, hc), adst_flat=adst.reshape(1, hc),
                    M_ae=M_ae, b=b.reshape(1, hc), H=h, C=c)

    layers = [pack_layer(i) for i in range(4)]

    # graph ranges per core for set2set (graphs never straddle cores)
    gbound = np.searchsorted(batch, np.arange(G + 1))  # node start of each graph id

    in_maps = []
    for cidx in range(NC):
        n0, n1 = cidx * NPC, (cidx + 1) * NPC
        e0, e1 = np.searchsorted(s_dst, n0), np.searchsorted(s_dst, n1)
        cs, cd, cea = s_src[e0:e1], s_dst[e0:e1] - n0, s_ea[e0:e1]

        # pack edges into W windows x K tiles x 128 slots
        idx_arr = np.zeros((W, P, K), np.int32)          # gather index (global node id)
        dcol = np.zeros((W, P, K), np.float32)           # dst rel to window (0..127)
        drow = np.zeros((W, K, P), np.float32)           # same, row-major per tile
        mask = np.zeros((W, P, K), np.float32)
        seaT = np.zeros((W, ED, K * P), np.float32)      # edge attr transposed per window
        for w in range(W):
            lo = w * P
            hi = min(lo + P, NPC)
            a = np.searchsorted(cd, lo)
            b2 = np.searchsorted(cd, hi)
            cnt = b2 - a
            assert cnt <= K * P, f"window overflow: core {cidx} win {w}: {cnt} > {K*P}"
            sl = slice(a, b2)
            # edge slot j -> (tile j // 128? ) we lay out edge j at tile=j%K? No:
            # tile k gets edges [k*128, (k+1)*128); within tile, partition = j-k*128
            js = np.arange(cnt)
            tk = js // P
            pp = js % P
            idx_arr[w, pp, tk] = cs[sl]
            dcol[w, pp, tk] = (cd[sl] - lo).astype(np.float32)
            drow[w, tk, pp] = (cd[sl] - lo).astype(np.float32)
            mask[w, pp, tk] = 1.0
            seaT[w, :, tk * P + pp] = cea[sl]

        # set2set: node range + padding for this core's graphs
        g0 = cidx * GPC
        gn0, gn1 = gbound[g0], gbound[g0 + GPC]
        ncnt = gn1 - gn0
        assert ncnt <= TS * P, f"s2s overflow core {cidx}: {ncnt} > {TS*P}"
        s2s_idx = np.zeros((TS * P,), np.int32)
        s2s_idx[:ncnt] = np.arange(gn0, gn1, dtype=np.int32)
        s2s_mask = np.zeros((TS * P,), np.float32)
        s2s_mask[:ncnt] = 1.0
        s2s_brel = np.zeros((TS * P,), np.float32)
        s2s_brel[:ncnt] = (batch[gn0:gn1] - g0).astype(np.float32)

        m = dict(
            xT=np.ascontiguousarray(x[n0:n1].T),                      # [ND, NPC]
            idx_in=idx_arr, dcol_in=dcol,
            drow_in=np.ascontiguousarray(drow.reshape(W, 1, K * P)),
            mask_in=mask, seaT_in=seaT,
            iota_in=np.broadcast_to(np.arange(P, dtype=np.float32), (P, P)).copy(),
            iotacol_in=np.arange(P, dtype=np.float32).reshape(P, 1),
            ones_in=np.ones((1, P), np.float32),
            s2s_idx_in=np.ascontiguousarray(s2s_idx.reshape(TS, P).T),
            s2s_mask_in=np.ascontiguousarray(s2s_mask.reshape(TS, P).T),
            s2s_brel_in=np.ascontiguousarray(s2s_brel.reshape(TS, P).T),
            s2s_brelr_in=s2s_brel.reshape(1, TS * P),
            gfT_in=np.ascontiguousarray(np.asarray(inp["global_features"], np.float32)[g0:g0 + GPC].T),  # [GD, GPC]
        )
        for i, L in enumerate(layers):
            m[f"W{i}"] = L["W"]
            m[f"asrcf{i}"] = np.broadcast_to(L["asrc_flat"], (P, D)).copy()
            m[f"adstf{i}"] = np.broadcast_to(L["adst_flat"], (P, D)).copy()
            m[f"Mae{i}"] = L["M_ae"]
            m[f"bias{i}"] = np.broadcast_to(L["b"], (P, D)).copy()
        # set2set weights: WihT [2D rows.., ..] transposed for matmul rhs
        m["WihT"] = np.ascontiguousarray(np.asarray(inp["s2s_Wih"], np.float32).T)   # [2D, 4D]
        m["WhhT"] = np.ascontiguousarray(np.asarray(inp["s2s_Whh"], np.float32).T)   # [D, 4D]
        m["s2s_bias"] = (np.asarray(inp["s2s_bih"], np.float32)
                         + np.asarray(inp["s2s_bhh"], np.float32)).reshape(1, -1)    # [1, 4D]
        m["p1W"] = np.asarray(inp["p1_W"], np.float32)     # [2D+GD, D]
        m["p1b"] = np.asarray(inp["p1_b"], np.float32).reshape(1, -1)
        m["p2W"] = np.asarray(inp["p2_W"], np.float32)
        m["p2b"] = np.asarray(inp["p2_b"], np.float32).reshape(1, -1)
        m["p3W"] = np.asarray(inp["p3_W"], np.float32)
        m["p3b"] = np.asarray(inp["p3_b"], np.float32).reshape(1, -1)
        in_maps.append(m)
    return in_maps


# ------------------------------------------------------------------
# device kernel builder
# ------------------------------------------------------------------

def build_kernel(cfg):
    N, NPC, W, K, GPC, TS = cfg.N, cfg.NPC, cfg.W, cfg.K, cfg.GPC, cfg.TS
    D, ED, GD = cfg.D, cfg.ED, cfg.GD
    TW = D + 8           # gather-table row width (lin 256 + a_src slot 8)
    HS = [8, 8, 8, 1]    # heads per layer
    DINS = [cfg.ND, D, D, D]
    NL = cfg.LAYERS
    STEPS = cfg.S2S_STEPS
    SB = K // 4          # rank-1 broadcast subbatches of 4 tiles

    nc = bacc.Bacc("TRN2", target_bir_lowering=False, debug=False,
                   num_devices=cfg.NC)

    # ---------------- inputs ----------------
    xT = nc.dram_tensor("xT", [cfg.ND, NPC], F32, kind="ExternalInput")
    idx_in = nc.dram_tensor("idx_in", [W, P, K], I32, kind="ExternalInput")
    dcol_in = nc.dram_tensor("dcol_in", [W, P, K], F32, kind="ExternalInput")
    drow_in = nc.dram_tensor("drow_in", [W, 1, K * P], F32, kind="ExternalInput")
    mask_in = nc.dram_tensor("mask_in", [W, P, K], F32, kind="ExternalInput")
    seaT_in = nc.dram_tensor("seaT_in", [W, ED, K * P], F32, kind="ExternalInput")
    iota_in = nc.dram_tensor("iota_in", [P, P], F32, kind="ExternalInput")
    iotacol_in = nc.dram_tensor("iotacol_in", [P, 1], F32, kind="ExternalInput")
    ones_in = nc.dram_tensor("ones_in", [1, P], F32, kind="ExternalInput")
    Wm, asrcf, adstf, Mae, biasg = [], [], [], [], []
    for i in range(NL):
        Wm.append(nc.dram_tensor(f"W{i}", [DINS[i], D], F32, kind="ExternalInput"))
        asrcf.append(nc.dram_tensor(f"asrcf{i}", [P, D], F32, kind="ExternalInput"))
        adstf.append(nc.dram_tensor(f"adstf{i}", [P, D], F32, kind="ExternalInput"))
        Mae.append(nc.dram_tensor(f"Mae{i}", [ED, HS[i]], F32, kind="ExternalInput"))
        biasg.append(nc.dram_tensor(f"bias{i}", [P, D], F32, kind="ExternalInput"))
    s2s_idx_in = nc.dram_tensor("s2s_idx_in", [P, TS], I32, kind="ExternalInput")
    s2s_mask_in = nc.dram_tensor("s2s_mask_in", [P, TS], F32, kind="ExternalInput")
    s2s_brel_in = nc.dram_tensor("s2s_brel_in", [P, TS], F32, kind="ExternalInput")
    s2s_brelr_in = nc.dram_tensor("s2s_brelr_in", [1, TS * P], F32, kind="ExternalInput")
    gfT_in = nc.dram_tensor("gfT_in", [GD, GPC], F32, kind="ExternalInput")
    WihT = nc.dram_tensor("WihT", [2 * D, 4 * D], F32, kind="ExternalInput")
    WhhT = nc.dram_tensor("WhhT", [D, 4 * D], F32, kind="ExternalInput")
    s2s_bias = nc.dram_tensor("s2s_bias", [1, 4 * D], F32, kind="ExternalInput")
    p1W = nc.dram_tensor("p1W", [2 * D + GD, D], F32, kind="ExternalInput")
    p1b = nc.dram_tensor("p1b", [1, D], F32, kind="ExternalInput")
    p2W = nc.dram_tensor("p2W", [D, D // 2], F32, kind="ExternalInput")
    p2b = nc.dram_tensor("p2b", [1, D // 2], F32, kind="ExternalInput")
    p3W = nc.dram_tensor("p3W", [D // 2, 5], F32, kind="ExternalInput")
    p3b = nc.dram_tensor("p3b", [1, 5], F32, kind="ExternalInput")
    out_t = nc.dram_tensor("out", [GPC, 5], F32, kind="ExternalOutput")

    T = dict(locals())
    with tile.TileContext(nc) as tc:
        build_body(nc, tc, cfg, T)
    nc.compile()
    return nc


def build_body(nc, tc, cfg, T):
    N, NPC, W, K, GPC, TS = cfg.N, cfg.NPC, cfg.W, cfg.K, cfg.GPC, cfg.TS
    D, ED, GD = cfg.D, cfg.ED, cfg.GD
    TW = D + 8
    HS = [8, 8, 8, 1]
    DINS = [cfg.ND, D, D, D]
    NL = cfg.LAYERS
    STEPS = cfg.S2S_STEPS
    SB = K // 4
    RG = [list(range(cfg.NC))]

    import contextlib
    ctx = contextlib.ExitStack()
    with ctx:
        pers = ctx.enter_context(tc.tile_pool(name="pers", bufs=1))
        dram = ctx.enter_context(tc.tile_pool(name="dram", bufs=1, space="DRAM"))

        # ---- persistent constants ----
        iota_sb = pers.tile([P, P], F32, tag="iota")
        nc.sync.dma_start(iota_sb[:], T["iota_in"][:])
        iotac_sb = pers.tile([P, 1], F32, tag="iotac")
        nc.sync.dma_start(iotac_sb[:], T["iotacol_in"][:])
        ones_sb = pers.tile([1, P], F32, tag="ones")
        nc.sync.dma_start(ones_sb[:], T["ones_in"][:])
        ident_sb = pers.tile([P, P], F32, tag="ident")
        nc.vector.tensor_tensor(out=ident_sb[:],
                                in0=iotac_sb[:].to_broadcast([P, P]),
                                in1=iota_sb[:], op=OP.is_equal)

        # transposed features ping-pong in DRAM (too big for SBUF at full scale)
        hT = [[dram.tile([P, W * P], F32, tag=f"hT{pp}_{c2}", name=f"hT{pp}_{c2}")
               for c2 in range(2)] for pp in range(2)]

        # DRAM scratch
        h_dram = [dram.tile([NPC, D], F32, tag=f"h{pp}", name=f"h{pp}")
                  for pp in range(2)]
        lin_local = dram.tile([NPC, TW], F32, tag="lin_local")
        tables = [dram.tile([N, TW], F32, tag=f"table{li}", name=f"table{li}",
                            addr_space="Shared") for li in range(NL)]
        hfin_local = dram.tile([NPC, D], F32, tag="hfin_local")
        hfin_table = dram.tile([N, D], F32, tag="hfin_table", addr_space="Shared")

        # ================= GAT layers =================
        with tc.tile_pool(name="lw", bufs=1) as lw, \
             tc.tile_pool(name="win", bufs=2) as win, \
             tc.tile_pool(name="psA", bufs=2, space="PSUM") as psA, \
             tc.tile_pool(name="psN", bufs=2, space="PSUM") as psN, \
             tc.tile_pool(name="psS", bufs=1, space="PSUM") as psS:
            for li in range(NL):
                H = HS[li]
                C = D // H
                din = DINS[li]
                nkc = (din + P - 1) // P   # contraction chunks for lin matmul

                # --- per-layer weights into SBUF ---
                W_sb = lw.tile([P, nkc * D], F32, tag="W_sb")
                for c2 in range(nkc):
                    r0, r1 = c2 * P, min((c2 + 1) * P, din)
                    nc.sync.dma_start(W_sb[: r1 - r0, c2 * D:(c2 + 1) * D],
                                      T["Wm"][li][r0:r1, :])
                asrc_b = lw.tile([P, D], F32, tag="asrc_b")
                nc.sync.dma_start(asrc_b[:], T["asrcf"][li][:])
                adst_b = lw.tile([P, D], F32, tag="adst_b")
                nc.sync.dma_start(adst_b[:], T["adstf"][li][:])
                bias_b = lw.tile([P, D], F32, tag="bias_b")
                nc.sync.dma_start(bias_b[:], T["biasg"][li][:])
                mae_sb = lw.tile([ED, 8], F32, tag="mae_sb")
                nc.sync.dma_start(mae_sb[:, :H], T["Mae"][li][:])
                adst_all = lw.tile([P, W * 8], F32, tag="adst_all")
                nc.vector.memset(adst_all[:], 0.0)

                hT_prev = hT[li % 2]
                hT_next = hT[(li + 1) % 2]
                h_prev = h_dram[(li + 1) % 2]
                h_next = h_dram[li % 2]

                # ---------- phase A: dense lin + a_src/a_dst ----------
                for w in range(W):
                    n0 = w * P
                    cnt = min(P, NPC - n0)
                    lin_ps = psA.tile([P, D], F32, tag="lin_ps")
                    if li == 0:
                        xTw = win.tile([cfg.ND, P], F32, tag="xTw")
                        nc.sync.dma_start(xTw[:, :cnt], T["xT"][:, n0:n0 + cnt])
                        nc.tensor.matmul(lin_ps[:cnt], lhsT=xTw[:, :cnt],
                                         rhs=W_sb[:din, 0:D], start=True, stop=True)
                    else:
                        hTw = win.tile([P, 2 * P], F32, tag="hTw")
                        for c2 in range(nkc):
                            nc.sync.dma_start(hTw[:, c2 * P:c2 * P + cnt],
                                              hT_prev[c2][:, n0:n0 + cnt])
                        for c2 in range(nkc):
                            nc.tensor.matmul(
                                lin_ps[:cnt],
                                lhsT=hTw[:, c2 * P:c2 * P + cnt],
                                rhs=W_sb[:, c2 * D:(c2 + 1) * D],
                                start=(c2 == 0), stop=(c2 == nkc - 1))
                    lin_sb = win.tile([P, D], F32, tag="lin_sb")
                    nc.vector.tensor_copy(lin_sb[:cnt], lin_ps[:cnt])
                    tmp = win.tile([P, D], F32, tag="tmpA")
                    nc.vector.tensor_tensor(out=tmp[:cnt], in0=lin_sb[:cnt],
                                            in1=asrc_b[:cnt], op=OP.mult)
                    a_s = win.tile([P, 8], F32, tag="a_s")
                    if H < 8:
                        nc.vector.memset(a_s[:], 0.0)
                    nc.vector.reduce_sum(
                        out=a_s[:cnt, :H],
                        in_=tmp[:cnt].rearrange("p (h c) -> p h c", h=H),
                        axis=mybir.AxisListType.X)
                    nc.vector.tensor_tensor(out=tmp[:cnt], in0=lin_sb[:cnt],
                                            in1=adst_b[:cnt], op=OP.mult)
                    nc.vector.reduce_sum(
                        out=adst_all[:cnt, w * 8:w * 8 + H],
                        in_=tmp[:cnt].rearrange("p (h c) -> p h c", h=H),
                        axis=mybir.AxisListType.X)
                    nc.sync.dma_start(lin_local[n0:n0 + cnt, 0:D], lin_sb[:cnt])
                    nc.sync.dma_start(lin_local[n0:n0 + cnt, D:D + 8], a_s[:cnt, :])

                # ---------- AllGather the transformed-feature table ----------
                if cfg.FAKE_AG:
                    nc.sync.dma_start(tables[li][0:NPC, :], lin_local[:])
                else:
                    nc.gpsimd.collective_compute(
                        "AllGather", OP.bypass, replica_groups=RG,
                        ins=[lin_local[:]], outs=[tables[li][:]])

                # ---------- phase B: per-window edge aggregation ----------
                for w in range(W):
                    n0 = w * P
                    cnt = min(P, NPC - n0)
                    idx_sb = win.tile([P, K], I32, tag="idx_sb")
                    nc.sync.dma_start(idx_sb[:], T["idx_in"][w])
                    dcol_sb = win.tile([P, K], F32, tag="dcol_sb")
                    nc.sync.dma_start(dcol_sb[:], T["dcol_in"][w])
                    drow_sb = win.tile([1, K * P], F32, tag="drow_sb")
                    nc.sync.dma_start(drow_sb[:], T["drow_in"][w])
                    mask_sb = win.tile([P, K], F32, tag="mask_sb")
                    nc.sync.dma_start(mask_sb[:], T["mask_in"][w])
                    seaT_sb = win.tile([ED, K * P], F32, tag="seaT_sb")
                    nc.sync.dma_start(seaT_sb[:], T["seaT_in"][w])

                    lin_g = win.tile([P, K, TW], F32, tag="lin_g")
                    for k in range(K):
                        nc.gpsimd.indirect_dma_start(
                            out=lin_g[:, k, :], out_offset=None, in_=tables[li][:],
                            in_offset=bass.IndirectOffsetOnAxis(
                                ap=idx_sb[:, k:k + 1], axis=0))

                    # indicator matrices: M (edges_p x nodes_f), MT (nodes_p x edges_f)
                    M_sb = win.tile([P, K, P], F32, tag="M_sb")
                    nc.vector.tensor_tensor(
                        out=M_sb[:],
                        in0=dcol_sb[:, :, None].to_broadcast([P, K, P]),
                        in1=iota_sb[:, None, :].to_broadcast([P, K, P]),
                        op=OP.is_equal)
                    MT_sb = win.tile([P, K * P], F32, tag="MT_sb")
                    for b in range(SB):
                        bc_ps = psA.tile([P, 4 * P], F32, tag="bc_ps")
                        nc.tensor.matmul(bc_ps[:], lhsT=ones_sb[:],
                                         rhs=drow_sb[:, b * 4 * P:(b + 1) * 4 * P],
                                         start=True, stop=True)
                        nc.vector.tensor_tensor(
                            out=MT_sb[:, b * 4 * P:(b + 1) * 4 * P],
                            in0=iotac_sb[:].to_broadcast([P, 4 * P]),
                            in1=bc_ps[:], op=OP.is_equal)

                    # alpha = lrelu(a_src + a_dst + a_edge); ex = exp * mask
                    al_ps = psS.tile([P, K * 8], F32, tag="al_ps")
                    for k in range(K):
                        nc.tensor.matmul(al_ps[:, k * 8:k * 8 + H],
                                         lhsT=MT_sb[:, k * P:(k + 1) * P],
                                         rhs=adst_all[:, w * 8:w * 8 + H],
                                         start=True, stop=False)
                        nc.tensor.matmul(al_ps[:, k * 8:k * 8 + H],
                                         lhsT=seaT_sb[:, k * P:(k + 1) * P],
                                         rhs=mae_sb[:, :H],
                                         start=False, stop=True)
                    al_sb = win.tile([P, K * 8], F32, tag="al_sb")
                    nc.vector.tensor_tensor(
                        out=al_sb[:, :K * H].rearrange("p (k h) -> p k h", k=K),
                        in0=al_ps[:].rearrange("p (k h) -> p k h", k=K)[:, :, :H],
                        in1=lin_g[:, :, D:D + H], op=OP.add)
                    # leaky_relu(x, 0.2) = max(0.2*x, x)
                    lr_sb = win.tile([P, K * 8], F32, tag="lr_sb")
                    nc.vector.tensor_scalar_mul(lr_sb[:, :K * H], al_sb[:, :K * H], 0.2)
                    nc.vector.tensor_tensor(out=lr_sb[:, :K * H], in0=lr_sb[:, :K * H],
                                            in1=al_sb[:, :K * H], op=OP.max)
                    ex_sb = win.tile([P, K * 8], F32, tag="ex_sb")
                    nc.scalar.activation(ex_sb[:, :K * H], lr_sb[:, :K * H], AF.Exp)
                    nc.vector.tensor_tensor(
                        out=ex_sb[:, :K * H].rearrange("p (k h) -> p k h", k=K),
                        in0=ex_sb[:, :K * H].rearrange("p (k h) -> p k h", k=K),
                        in1=mask_sb[:, :, None].to_broadcast([P, K, H]),
                        op=OP.mult)

                    # weighted features + segment sums
                    nu_ps = psN.tile([P, D + 8], F32, tag="nu_ps")
                    for b in range(SB):
                        wfex = win.tile([P, 4, D + 8], F32, tag="wfex")
                        nc.vector.tensor_tensor(
                            out=wfex[:, :, 0:D].rearrange(
                                "p k (h c) -> p k h c", h=H),
                            in0=lin_g[:, b * 4:(b + 1) * 4, 0:D].rearrange(
                                "p k (h c) -> p k h c", h=H),
                            in1=ex_sb[:, b * 4 * H:(b + 1) * 4 * H].rearrange(
                                "p (k h) -> p k h", k=4)[:, :, :, None].to_broadcast(
                                [P, 4, H, C]),
                            op=OP.mult)
                        nc.vector.tensor_copy(
                            wfex[:, :, D:D + H],
                            ex_sb[:, b * 4 * H:(b + 1) * 4 * H].rearrange(
                                "p (k h) -> p k h", k=4))
                        for kk in range(4):
                            k = b * 4 + kk
                            nc.tensor.matmul(
                                nu_ps[:, 0:D + H],
                                lhsT=M_sb[:, k, :],
                                rhs=wfex[:, kk, 0:D + H],
                                start=(k == 0), stop=(k == K - 1))

                    # normalize, bias, ELU, residual
                    den = win.tile([P, 8], F32, tag="den")
                    nc.vector.tensor_scalar_add(den[:cnt, :H], nu_ps[:cnt, D:D + H], 1e-16)
                    rec = win.tile([P, 8], F32, tag="rec")
                    nc.vector.reciprocal(rec[:cnt, :H], den[:cnt, :H])
                    outw = win.tile([P, D], F32, tag="outw")
                    nc.vector.tensor_tensor(
                        out=outw[:cnt].rearrange("p (h c) -> p h c", h=H),
                        in0=nu_ps[:cnt, 0:D].rearrange("p (h c) -> p h c", h=H),
                        in1=rec[:cnt, :H][:, :, None].to_broadcast([cnt, H, C]),
                        op=OP.mult)
                    nc.vector.tensor_tensor(out=outw[:cnt], in0=outw[:cnt],
                                            in1=bias_b[:cnt], op=OP.add)
                    # ELU = relu(x) + exp(min(x,0)) - 1
                    tmin = win.tile([P, D], F32, tag="tmin")
                    nc.vector.tensor_scalar_min(tmin[:cnt], outw[:cnt], 0.0)
                    nc.scalar.activation(tmin[:cnt], tmin[:cnt], AF.Exp)
                    trel = win.tile([P, D], F32, tag="trel")
                    nc.vector.tensor_scalar_max(trel[:cnt], outw[:cnt], 0.0)
                    hn = win.tile([P, D], F32, tag="hn")
                    if cnt < P:
                        nc.vector.memset(hn[:], 0.0)
                    nc.vector.tensor_tensor(out=hn[:cnt], in0=tmin[:cnt],
                                            in1=trel[:cnt], op=OP.add)
                    nc.vector.tensor_scalar_add(hn[:cnt], hn[:cnt], -1.0)
                    if li > 0:
                        hp = win.tile([P, D], F32, tag="hp")
                        nc.sync.dma_start(hp[:cnt], h_prev[n0:n0 + cnt])
                        nc.vector.tensor_tensor(out=hn[:cnt], in0=hn[:cnt],
                                                in1=hp[:cnt], op=OP.add)
                    if li < NL - 1:
                        nc.sync.dma_start(h_next[n0:n0 + cnt], hn[:cnt])
                        for c2 in range(2):
                            tr_ps = psS.tile([P, P], F32, tag="tr_ps")
                            nc.tensor.transpose(tr_ps[:], hn[:, c2 * P:(c2 + 1) * P],
                                                ident_sb[:])
                            trc = win.tile([P, P], F32, tag="trc")
                            nc.vector.tensor_copy(trc[:, :cnt], tr_ps[:, :cnt])
                            nc.sync.dma_start(hT_next[c2][:, n0:n0 + cnt],
                                              trc[:, :cnt])
                    else:
                        nc.sync.dma_start(hfin_local[n0:n0 + cnt], hn[:cnt])

            # final AllGather of node features for set2set
            if cfg.FAKE_AG:
                nc.sync.dma_start(hfin_table[0:NPC, :], hfin_local[:])
            else:
                nc.gpsimd.collective_compute(
                    "AllGather", OP.bypass, replica_groups=RG,
                    ins=[hfin_local[:]], outs=[hfin_table[:]])

        # ================= Set2Set + MLP head =================
        build_s2s(nc, tc, cfg, T, pers, dram, hfin_table,
                  iota_sb, iotac_sb, ones_sb, ident_sb)


def build_s2s(nc, tc, cfg, T, pers, dram, hfin_table,
              iota_sb, iotac_sb, ones_sb, ident_sb):
    N, NPC, GPC, TS = cfg.N, cfg.NPC, cfg.GPC, cfg.TS
    D, GD = cfg.D, cfg.GD
    GG = GPC
    STEPS = cfg.S2S_STEPS

    with tc.tile_pool(name="s2s", bufs=1) as sp, \
         tc.tile_pool(name="ps2", bufs=1, space="PSUM") as ps2:
        # gather this core's node features (padded to TS*128)
        s2s_idx = sp.tile([P, TS], I32, tag="s2s_idx")
        nc.sync.dma_start(s2s_idx[:], T["s2s_idx_in"][:])
        xn = sp.tile([P, TS, D], F32, tag="xn")
        for t in range(TS):
            nc.gpsimd.indirect_dma_start(
                out=xn[:, t, :], out_offset=None, in_=hfin_table[:],
                in_offset=bass.IndirectOffsetOnAxis(ap=s2s_idx[:, t:t + 1], axis=0))
        maskc = sp.tile([P, TS], F32, tag="maskc")
        nc.sync.dma_start(maskc[:], T["s2s_mask_in"][:])
        brelc = sp.tile([P, TS], F32, tag="brelc")
        nc.sync.dma_start(brelc[:], T["s2s_brel_in"][:])
        brelr = sp.tile([1, TS * P], F32, tag="brelr")
        nc.sync.dma_start(brelr[:], T["s2s_brelr_in"][:])

        # indicator matrices per node tile (once for all steps)
        Mb = sp.tile([P, TS * GG], F32, tag="Mb")       # node_p x graph_f
        nc.vector.tensor_tensor(
            out=Mb[:].rearrange("p (t g) -> p t g", t=TS),
            in0=brelc[:, :, None].to_broadcast([P, TS, GG]),
            in1=iota_sb[:, None, 0:GG].to_broadcast([P, TS, GG]),
            op=OP.is_equal)
        MbT = sp.tile([GG, TS * P], F32, tag="MbT")     # graph_p x node_f
        for t in range(TS):
            bc_ps = ps2.tile([GG, P], F32, tag="psX")
            nc.tensor.matmul(bc_ps[:], lhsT=ones_sb[:, 0:GG],
                             rhs=brelr[:, t * P:(t + 1) * P], start=True, stop=True)
            nc.vector.tensor_tensor(
                out=MbT[:, t * P:(t + 1) * P],
                in0=iotac_sb[:GG].to_broadcast([GG, P]),
                in1=bc_ps[:], op=OP.is_equal)

        # s2s weights: WihT [2D,4D] in 4 row-chunks, WhhT [D,4D] in 2 row-chunks
        wih = sp.tile([P, 4 * 4 * D], F32, tag="wih")
        for c2 in range(4):
            nc.sync.dma_start(wih[:, c2 * 4 * D:(c2 + 1) * 4 * D],
                              T["WihT"][c2 * P:(c2 + 1) * P, :])
        whh = sp.tile([P, 2 * 4 * D], F32, tag="whh")
        for c2 in range(2):
            nc.sync.dma_start(whh[:, c2 * 4 * D:(c2 + 1) * 4 * D],
                              T["WhhT"][c2 * P:(c2 + 1) * P, :])
        s2sb = sp.tile([1, 4 * D], F32, tag="s2sb")
        nc.sync.dma_start(s2sb[:], T["s2s_bias"][:])

        # LSTM / attention state: q*^T chunks (h part then r part), h^T chunks, c
        qT = [sp.tile([P, GG], F32, tag=f"qT{c2}", name=f"qT{c2}") for c2 in range(4)]
        c_st = sp.tile([GG, D], F32, tag="c_st")
        for t_ in qT:
            nc.vector.memset(t_[:], 0.0)
        nc.vector.memset(c_st[:], 0.0)

        gact = [AF.Sigmoid, AF.Sigmoid, AF.Tanh, AF.Sigmoid]  # i, f, g, o
        for step in range(STEPS):
            gs = []
            for g in range(4):
                g_ps = ps2.tile([GG, D], F32, tag="psY")
                nc.tensor.matmul(g_ps[:], lhsT=ones_sb[:, 0:GG],
                                 rhs=s2sb[:, g * D:(g + 1) * D],
                                 start=True, stop=False)
                for c2 in range(4):
                    nc.tensor.matmul(
                        g_ps[:], lhsT=qT[c2][:],
                        rhs=wih[:, c2 * 4 * D + g * D: c2 * 4 * D + (g + 1) * D],
                        start=False, stop=False)
                for c2 in range(2):
                    # h part of q_star doubles as the LSTM h for Whh
                    nc.tensor.matmul(
                        g_ps[:], lhsT=qT[c2][:],
                        rhs=whh[:, c2 * 4 * D + g * D: c2 * 4 * D + (g + 1) * D],
                        start=False, stop=(c2 == 1))
                g_sb = sp.tile([GG, D], F32, tag=f"g_sb{g}")
                nc.scalar.activation(g_sb[:], g_ps[:], gact[g])
                gs.append(g_sb)
            # c = f*c + i*tanh(g);  h = o*tanh(c)
            t1 = sp.tile([GG, D], F32, tag="t1")
            nc.vector.tensor_tensor(out=t1[:], in0=gs[0][:], in1=gs[2][:], op=OP.mult)
            nc.vector.tensor_tensor(out=c_st[:], in0=gs[1][:], in1=c_st[:], op=OP.mult)
            nc.vector.tensor_tensor(out=c_st[:], in0=c_st[:], in1=t1[:], op=OP.add)
            tc_sb = sp.tile([GG, D], F32, tag="tc_sb")
            nc.scalar.activation(tc_sb[:], c_st[:], AF.Tanh)
            h_l = sp.tile([GG, D], F32, tag="h_l")
            nc.vector.tensor_tensor(out=h_l[:], in0=gs[3][:], in1=tc_sb[:], op=OP.mult)

            # attention over nodes: e = <xn, h[batch]>, softmax per graph
            e_all = sp.tile([P, TS], F32, tag="e_all")
            escr = sp.tile([P, D], F32, tag="escr")
            for t in range(TS):
                he_ps = ps2.tile([P, D], F32, tag="psH")
                nc.tensor.matmul(he_ps[:], lhsT=MbT[:, t * P:(t + 1) * P],
                                 rhs=h_l[:], start=True, stop=True)
                nc.vector.tensor_tensor(out=escr[:], in0=xn[:, t, :],
                                        in1=he_ps[:], op=OP.mult)
                nc.vector.reduce_sum(out=e_all[:, t:t + 1], in_=escr[:],
                                     axis=mybir.AxisListType.X)
            nc.scalar.activation(e_all[:], e_all[:], AF.Exp)
            nc.vector.tensor_tensor(out=e_all[:], in0=e_all[:], in1=maskc[:],
                                    op=OP.mult)
            r_ps = ps2.tile([GG, D + 1], F32, tag="psR")
            for t in range(TS):
                wxex = sp.tile([P, D + 1], F32, tag="wxex")
                nc.vector.tensor_tensor(
                    out=wxex[:, 0:D], in0=xn[:, t, :],
                    in1=e_all[:, t:t + 1].to_broadcast([P, D]), op=OP.mult)
                nc.vector.tensor_copy(wxex[:, D:D + 1], e_all[:, t:t + 1])
                nc.tensor.matmul(r_ps[:], lhsT=Mb[:, t * GG:(t + 1) * GG],
                                 rhs=wxex[:], start=(t == 0), stop=(t == TS - 1))
            den = sp.tile([GG, 1], F32, tag="s2s_den")
            nc.vector.tensor_scalar_add(den[:], r_ps[:, D:D + 1], 1e-16)
            rec = sp.tile([GG, 1], F32, tag="s2s_rec")
            nc.vector.reciprocal(rec[:], den[:])
            r_sb = sp.tile([GG, D], F32, tag="r_sb")
            nc.vector.tensor_tensor(out=r_sb[:], in0=r_ps[:, 0:D],
                                    in1=rec[:].to_broadcast([GG, D]), op=OP.mult)
            # q_star^T = [h^T | r^T] for next step / head
            for c2 in range(2):
                tr_ps = ps2.tile([P, GG], F32, tag="psX")
                nc.tensor.transpose(tr_ps[:], h_l[:, c2 * P:(c2 + 1) * P],
                                    ident_sb[:GG, :GG])
                nc.vector.tensor_copy(qT[c2][:], tr_ps[:])
                tr_ps2 = ps2.tile([P, GG], F32, tag="psX")
                nc.tensor.transpose(tr_ps2[:], r_sb[:, c2 * P:(c2 + 1) * P],
                                    ident_sb[:GG, :GG])
                nc.vector.tensor_copy(qT[2 + c2][:], tr_ps2[:])

        # ---------------- MLP head ----------------
        gfT_sb = sp.tile([GD, GG], F32, tag="gfT_sb")
        nc.sync.dma_start(gfT_sb[:], T["gfT_in"][:])
        p1w_sb = sp.tile([P, 4 * D], F32, tag="p1w_sb")
        for c2 in range(4):
            nc.sync.dma_start(p1w_sb[:, c2 * D:(c2 + 1) * D],
                              T["p1W"][c2 * P:(c2 + 1) * P, :])
        p1wg_sb = sp.tile([GD, D], F32, tag="p1wg_sb")
        nc.sync.dma_start(p1wg_sb[:], T["p1W"][4 * P:4 * P + GD, :])
        p1b_sb = sp.tile([1, D], F32, tag="p1b_sb")
        nc.sync.dma_start(p1b_sb[:], T["p1b"][:])
        z1_ps = ps2.tile([GG, D], F32, tag="psY")
        nc.tensor.matmul(z1_ps[:], lhsT=ones_sb[:, 0:GG], rhs=p1b_sb[:],
                         start=True, stop=False)
        for c2 in range(4):
            nc.tensor.matmul(z1_ps[:], lhsT=qT[c2][:],
                             rhs=p1w_sb[:, c2 * D:(c2 + 1) * D],
                             start=False, stop=False)
        nc.tensor.matmul(z1_ps[:], lhsT=gfT_sb[:], rhs=p1wg_sb[:],
                         start=False, stop=True)
        z1 = sp.tile([GG, D], F32, tag="z1")
        nc.scalar.activation(z1[:], z1_ps[:], AF.Relu)

        p2w_sb = sp.tile([P, 2 * (D // 2)], F32, tag="p2w_sb")
        for c2 in range(2):
            nc.sync.dma_start(p2w_sb[:, c2 * (D // 2):(c2 + 1) * (D // 2)],
                              T["p2W"][c2 * P:(c2 + 1) * P, :])
        p2b_sb = sp.tile([1, D // 2], F32, tag="p2b_sb")
        nc.sync.dma_start(p2b_sb[:], T["p2b"][:])
        z2_ps = ps2.tile([GG, D // 2], F32, tag="psY")
        nc.tensor.matmul(z2_ps[:], lhsT=ones_sb[:, 0:GG], rhs=p2b_sb[:],
                         start=True, stop=False)
        for c2 in range(2):
            z1T_ps = ps2.tile([P, GG], F32, tag="psX")
            nc.tensor.transpose(z1T_ps[:], z1[:, c2 * P:(c2 + 1) * P], ident_sb[:GG, :GG])
            z1T = sp.tile([P, GG], F32, tag="z1T")
            nc.vector.tensor_copy(z1T[:], z1T_ps[:])
            nc.tensor.matmul(z2_ps[:], lhsT=z1T[:],
                             rhs=p2w_sb[:, c2 * (D // 2):(c2 + 1) * (D // 2)],
                             start=False, stop=(c2 == 1))
        z2 = sp.tile([GG, D // 2], F32, tag="z2")
        nc.scalar.activation(z2[:], z2_ps[:], AF.Relu)

        p3w_sb = sp.tile([D // 2, 5], F32, tag="p3w_sb")
        nc.sync.dma_start(p3w_sb[:], T["p3W"][:])
        p3b_sb = sp.tile([1, 5], F32, tag="p3b_sb")
        nc.sync.dma_start(p3b_sb[:], T["p3b"][:])
        z2T_ps = ps2.tile([P, GG], F32, tag="psX")
        nc.tensor.transpose(z2T_ps[:], z2[:], ident_sb[:GG, :GG])
        z2T = sp.tile([P, GG], F32, tag="z2T")
        nc.vector.tensor_copy(z2T[:], z2T_ps[:])
        o_ps = ps2.tile([GG, 5], F32, tag="psY")
        nc.tensor.matmul(o_ps[:], lhsT=ones_sb[:, 0:GG], rhs=p3b_sb[:],
                         start=True, stop=False)
        nc.tensor.matmul(o_ps[:], lhsT=z2T[:], rhs=p3w_sb[:],
                         start=False, stop=True)
        o_sb = sp.tile([GG, 5], F32, tag="o_sb")
        nc.vector.tensor_copy(o_sb[:], o_ps[:])
        nc.sync.dma_start(T["out_t"][:], o_sb[:cfg.GPC])


def run_config(inputs, cfg):
    in_maps = host_prep(inputs, cfg)
    nc = build_kernel(cfg)
    res = run_bass_kernel_spmd(nc, in_maps, core_ids=list(range(cfg.NC)))
    out = np.concatenate([res.results[c]["out"] for c in range(cfg.NC)], axis=0)
    return out.astype(np.float32)


def kernel(**inputs):
    return run_config(inputs, CFG.derive())



# revision 77
# speedup vs baseline: 1.8489x; 1.8489x over previous
"""Trainium2 Bass kernel for AdvancedGATModel (4-layer edge-featured GAT +
Set2Set pooling + MLP head), sharded across 8 NeuronCores.

Sharding: nodes are split into 8 contiguous slices (6250 each); each core owns
the edges whose *destination* lands in its slice (plus self-loops), so segment
softmax and the scatter-add aggregation are core-local.  Per layer each core
computes the linear transform of its node slice, all cores AllGather the
transformed features (bf16) into a replicated table, and each core gathers its
edges' source rows via batched indirect DMA.  Segment softmax/weighted-sum run
as dense 128-edge-tile bf16 matmuls against on-the-fly one-hot "indicator"
matrices (edges x window-nodes).  Set2Set is sharded by graph (64 graphs/core;
graphs never straddle cores), and the [64,5] head outputs are concatenated on
host.

Perf notes vs the original version:
 - everything feeding the tensor engine is bf16 (4x column rate, fp32 PSUM
   accumulate), halves gather/AllGather bytes;
 - indirect gathers are batched (<=7 edge-tiles per SWDGE instruction instead
   of 1) to amortize the ~1us fixed SWDGE cost;
 - per-edge metadata (idx/dcol/mask) is loaded into SBUF once for the whole
   kernel; drow+edge-attrs are one DMA per window;
 - hT (transposed features) and the residual h live in SBUF across layers, so
   phase A reads weights straight from SBUF and phase B's residual add needs
   no DMA;
 - per-window edge-tile count K_w is the max over cores of what's needed
   (instead of a global worst case).

The program is identical on all 8 cores (SPMD); only input *data* differs.
All shapes below are hardcoded for the grading problem.
"""

import numpy as np
import ml_dtypes

import concourse.bass as bass
import concourse.bacc as bacc
import concourse.tile as tile
import concourse.mybir as mybir
from concourse.bass_utils import run_bass_kernel_spmd

F32 = mybir.dt.float32
BF16 = mybir.dt.bfloat16
I32 = mybir.dt.int32
I16 = mybir.dt.int16
AF = mybir.ActivationFunctionType
OP = mybir.AluOpType
P = 128
BF = ml_dtypes.bfloat16
TWP = 384       # gather-table row, padded to 256B-multiples for dma_gather


class CFG:
    # full problem; small-mode tests override these
    N = 50000          # nodes
    E = 800000         # edges (before self loops)
    G = 512            # graphs
    ND = 14            # node feat dim
    ED = 4             # edge feat dim
    GD = 13            # global feat dim
    D = 256            # hidden
    H = 8              # heads
    C = 32             # per-head channels
    NC = 8             # cores
    NPC = N // NC      # nodes per core = 6250
    W = 49             # node windows per core (ceil(NPC/128))
    GPC = G // NC      # graphs per core = 64
    TS = 51            # set2set node tiles per core (capacity TS*128 nodes)
    S2S_STEPS = 3
    LAYERS = 4
    SPLIT = 32768       # dma_gather int16 index limit: table split point
    FAKE_AG = False     # replace AllGathers with local copies (debug only)
    DBG = False         # dump per-layer tables as extra outputs (debug only)

    @classmethod
    def derive(cls):
        cls.NPC = cls.N // cls.NC
        cls.GPC = cls.G // cls.NC
        cls.W = -(-cls.NPC // P)
        return cls


def make_small_cfg():
    class Small(CFG):
        N = 2048
        E = 8192
        G = 64
        TS = 3
        SPLIT = 1024   # exercise the two-group gather path at small scale
    return Small.derive()


# ------------------------------------------------------------------
# host-side preprocessing
# ------------------------------------------------------------------

def host_prep(inp, cfg):
    """Build per-core input maps from the full input dict.

    Returns (in_maps, pack) where pack carries the SPMD-uniform edge packing
    (per-window tile counts) that build_kernel hardcodes into the program.
    """
    N, E, G = cfg.N, cfg.E, cfg.G
    NC, NPC, W, GPC, TS = cfg.NC, cfg.NPC, cfg.W, cfg.GPC, cfg.TS
    D, H, C, ED, GD = cfg.D, cfg.H, cfg.C, cfg.ED, cfg.GD

    src = np.asarray(inp["edge_index"][0])
    dst = np.asarray(inp["edge_index"][1])
    ea = np.asarray(inp["edge_attr"], dtype=np.float32)
    batch = np.asarray(inp["batch_idx"])
    x = np.asarray(inp["x"], dtype=np.float32)

    # self-loop attr = mean incoming edge attr (0 for isolated nodes)
    deg = np.bincount(dst, minlength=N).astype(np.float32)
    loop = np.zeros((N, ED), np.float32)
    for j in range(ED):
        loop[:, j] = np.bincount(dst, weights=ea[:, j], minlength=N)
    loop /= np.maximum(deg, 1.0)[:, None]

    src2 = np.concatenate([src, np.arange(N, dtype=np.int64)])
    dst2 = np.concatenate([dst, np.arange(N, dtype=np.int64)])
    ea2 = np.concatenate([ea, loop], axis=0).astype(np.float32)

    order = np.argsort(dst2, kind="stable")
    s_src = src2[order]
    s_dst = dst2[order]
    s_ea = ea2[order]

    # per-core edge slices; per-window counts split by table row < SPLIT
    # (group A) vs >= SPLIT (group B) for the int16-indexed dma_gather.
    # Table rows are laid out [core][p][w] (partition-major within a core) so
    # phase A can store slabs with one contiguous DMA per partition:
    #   src node s -> row  c*P*W + p*W + w   (c=s//NPC, w=(s%NPC)//P, p=s%P)
    SPLIT = cfg.SPLIT
    def src2row(s):
        c, m = s // NPC, s % NPC
        return c * P * W + (m % P) * W + m // P
    core_groups = []
    cntA = np.zeros((NC, W), np.int64)
    cntB = np.zeros((NC, W), np.int64)
    for cidx in range(NC):
        n0, n1 = cidx * NPC, (cidx + 1) * NPC
        e0, e1 = np.searchsorted(s_dst, n0), np.searchsorted(s_dst, n1)
        cs, cd, cea = s_src[e0:e1], s_dst[e0:e1] - n0, s_ea[e0:e1]
        per_w = []
        for w in range(W):
            lo, hi = w * P, min((w + 1) * P, NPC)
            a = np.searchsorted(cd, lo)
            b2 = np.searchsorted(cd, hi)
            sl = slice(a, b2)
            rows = src2row(cs[sl])
            gA = rows < SPLIT
            per_w.append((rows, (cd[sl] - lo).astype(np.float32),
                          cea[sl], gA))
            cntA[cidx, w] = gA.sum()
            cntB[cidx, w] = (~gA).sum()
        core_groups.append(per_w)
    TA_w = [max(1, int(-(-cntA[:, w].max() // P))) for w in range(W)]
    TB_w = [int(-(-cntB[:, w].max() // P)) for w in range(W)]
    K_w = [TA_w[w] + TB_w[w] for w in range(W)]
    KOFF = np.concatenate([[0], np.cumsum(K_w)]).astype(np.int64)
    TK = int(KOFF[-1])
    KMAX = max(K_w)
    pack = dict(K_w=K_w, TA_w=TA_w, TB_w=TB_w, KOFF=KOFF, TK=TK, KMAX=KMAX)
    # filled below once weights are inspected
    pack["use_bias"] = False

    # per-layer host-packed weight helpers.
    # Features are stored CHANNEL-MAJOR on device: new col (c,h) = c*H8 + h
    # with H8=8 (layer 3's H=1 output adopts the same permutation so the
    # residual add stays elementwise-consistent).  All weight matrices are
    # permuted here on the host; a_src/a_dst are folded into the lin matmul
    # as two extra 8-wide column blocks Wa = W @ asrcBD, Wd = W @ adstBD.
    PERM = (np.arange(8)[None, :] * (D // 8)
            + np.arange(D // 8)[:, None]).flatten()          # [D] new->old

    def pack_layer(i):
        Wm = np.asarray(inp[f"g{i}_W"], np.float32)          # [din, H*C]
        We = np.asarray(inp[f"g{i}_We"], np.float32)         # [ED, H*C]
        asrc = np.asarray(inp[f"g{i}_asrc"], np.float32)     # [h, c]
        adst = np.asarray(inp[f"g{i}_adst"], np.float32)
        aedge = np.asarray(inp[f"g{i}_aedge"], np.float32)
        b = np.asarray(inp[f"g{i}_b"], np.float32)
        h, c = asrc.shape
        hc = h * c
        # M_ae[d, h] = sum_c We[d, h*c+cc] * aedge[h, cc]
        M_ae = (We.reshape(ED, h, c) * aedge[None]).sum(-1)  # [ED, h]
        # a_src[n, :] = h_in[n] @ Wa  (block-diag contraction over c)
        asrcBD = np.zeros((hc, 8), np.float32)
        adstBD = np.zeros((hc, 8), np.float32)
        for hh in range(h):
            asrcBD[hh * c:(hh + 1) * c, hh] = asrc[hh]
            adstBD[hh * c:(hh + 1) * c, hh] = adst[hh]
        Wa = Wm @ asrcBD                                     # [din, 8]
        Wd = Wm @ adstBD
        Wp = Wm[:, PERM]                                     # output c-major
        bp = b[PERM]
        if i > 0:                                            # input also perm
            Wp = Wp[PERM]
            Wa = Wa[PERM]
            Wd = Wd[PERM]
        Wfull = np.concatenate([Wp, Wa, Wd], axis=1)         # [din, D+16]
        return dict(W=Wfull, M_ae=M_ae, b=bp.reshape(1, hc), H=h, C=c)

    layers = [pack_layer(i) for i in range(4)]
    pack["use_bias"] = bool(any(np.abs(np.asarray(inp[f"g{i}_b"])).max() > 0
                                for i in range(4)))

    # graph ranges per core for set2set (graphs never straddle cores)
    gbound = np.searchsorted(batch, np.arange(G + 1))  # node start of each graph

    in_maps = []
    for cidx in range(NC):
        n0, n1 = cidx * NPC, (cidx + 1) * NPC

        # flat edge packing: window w owns tiles KOFF[w]..KOFF[w+1];
        # group A (row < SPLIT) fills tiles [0, TA), group B [TA, TA+TB)
        idx16 = np.zeros((P, 8 * TK), np.int16)    # [16-band replicated x8]
        dcol_fl = np.zeros((P, TK), BF)            # dst rel to window (0..127)
        mask_fl = np.zeros((P, TK), BF)
        seaT_fl = np.zeros((ED, TK * P), BF)       # edge attrs, transposed
        for w in range(W):
            rows_w, rel_w, cea, gA = core_groups[cidx][w]
            for base, t0, sel in ((0, 0, gA), (SPLIT, TA_w[w], ~gA)):
                es = rows_w[sel]
                rel = rel_w[sel]
                ea = cea[sel]
                cnt = len(es)
                if cnt == 0:
                    continue
                js = np.arange(cnt)
                tk = KOFF[w] + t0 + js // P
                pp = js % P
                dcol_fl[pp, tk] = rel.astype(BF)
                mask_fl[pp, tk] = 1.0
                seaT_fl[:, tk * P + pp] = ea.T.astype(BF)
                cols = 8 * (KOFF[w] + t0) + js // 16
                idx16[js % 16, cols] = (es - base).astype(np.int16)
        for band in range(1, 8):
            idx16[16 * band:16 * (band + 1)] = idx16[:16]

        # set2set: node range + padding for this core's graphs
        g0 = cidx * GPC
        gn0, gn1 = gbound[g0], gbound[g0 + GPC]
        ncnt = gn1 - gn0
        assert ncnt <= TS * P, f"s2s overflow core {cidx}: {ncnt} > {TS*P}"
        s2s_idx = np.zeros((TS * P,), np.int32)
        s2s_idx[:ncnt] = np.arange(gn0, gn1, dtype=np.int32)
        s2s_mask = np.zeros((TS * P,), np.float32)
        s2s_mask[:ncnt] = 1.0
        s2s_brel = np.zeros((TS * P,), np.float32)
        s2s_brel[:ncnt] = (batch[gn0:gn1] - g0).astype(np.float32)

        m = dict(
            xT=np.ascontiguousarray(x[n0:n1].T).astype(BF),           # [ND, NPC]
            idx16_in=idx16, dcol_in=dcol_fl, mask_in=mask_fl, srow_in=seaT_fl,
            iota_in=np.broadcast_to(np.arange(P, dtype=np.float32),
                                    (P, P)).astype(BF).copy(),
            iotacol_in=np.arange(P, dtype=np.float32).reshape(P, 1).astype(BF),
            ones_in=np.ones((1, P), BF),
            s2s_idx_in=np.ascontiguousarray(s2s_idx.reshape(TS, P).T),
            s2s_mask_in=np.ascontiguousarray(s2s_mask.reshape(TS, P).T),
            s2s_brel_in=np.ascontiguousarray(s2s_brel.reshape(TS, P).T).astype(BF),
            s2s_brelr_in=s2s_brel.reshape(1, TS * P).astype(BF),
            gfT_in=np.ascontiguousarray(
                np.asarray(inp["global_features"], np.float32)[g0:g0 + GPC].T
            ).astype(BF),                                             # [GD, GPC]
        )
        for i, L in enumerate(layers):
            m[f"W{i}"] = L["W"].astype(BF)
            m[f"Mae{i}"] = L["M_ae"].astype(BF)
            m[f"bias{i}"] = np.broadcast_to(L["b"], (P, D)).copy()   # f32
        # s2s weights follow the c-major feature permutation: q_star rows and
        # per-gate output columns permute so <xn, h> stays consistent
        qperm = np.concatenate([PERM, PERM + D])
        gperm = np.concatenate([g * D + PERM for g in range(4)])
        WihT_p = np.asarray(inp["s2s_Wih"], np.float32).T[qperm][:, gperm]
        WhhT_p = np.asarray(inp["s2s_Whh"], np.float32).T[PERM][:, gperm]
        m["WihT"] = np.ascontiguousarray(WihT_p).astype(BF)          # [2D, 4D]
        m["WhhT"] = np.ascontiguousarray(WhhT_p).astype(BF)          # [D, 4D]
        m["s2s_bias"] = (np.asarray(inp["s2s_bih"], np.float32)
                         + np.asarray(inp["s2s_bhh"], np.float32)
                         )[gperm].reshape(1, -1).astype(BF)          # [1, 4D]
        p1_rows = np.concatenate([qperm, np.arange(2 * D, 2 * D + GD)])
        m["p1W"] = np.asarray(inp["p1_W"], np.float32)[p1_rows].astype(BF)
        m["p1b"] = np.asarray(inp["p1_b"], np.float32).reshape(1, -1).astype(BF)
        m["p2W"] = np.asarray(inp["p2_W"], np.float32).astype(BF)
        m["p2b"] = np.asarray(inp["p2_b"], np.float32).reshape(1, -1).astype(BF)
        m["p3W"] = np.asarray(inp["p3_W"], np.float32).astype(BF)
        m["p3b"] = np.asarray(inp["p3_b"], np.float32).reshape(1, -1).astype(BF)
        in_maps.append(m)
    return in_maps, pack


# ------------------------------------------------------------------
# device kernel builder
# ------------------------------------------------------------------

def build_kernel(cfg, pack, reps=1):
    N, NPC, W, GPC, TS = cfg.N, cfg.NPC, cfg.W, cfg.GPC, cfg.TS
    D, ED, GD = cfg.D, cfg.ED, cfg.GD
    TW = D + 8           # gather-table row width (lin 256 + a_src slot 8)
    HS = [8, 8, 8, 1]    # heads per layer
    DINS = [cfg.ND, D, D, D]
    NL = cfg.LAYERS
    TK = pack["TK"]

    nc = bacc.Bacc("TRN2", target_bir_lowering=False, debug=False,
                   num_devices=cfg.NC, num_swdge_queues=4)

    # ---------------- inputs ----------------
    xT = nc.dram_tensor("xT", [cfg.ND, NPC], BF16, kind="ExternalInput")
    idx16_in = nc.dram_tensor("idx16_in", [P, 8 * TK], I16, kind="ExternalInput")
    dcol_in = nc.dram_tensor("dcol_in", [P, TK], BF16, kind="ExternalInput")
    mask_in = nc.dram_tensor("mask_in", [P, TK], BF16, kind="ExternalInput")
    srow_in = nc.dram_tensor("srow_in", [ED, TK * P], BF16,
                             kind="ExternalInput")
    iota_in = nc.dram_tensor("iota_in", [P, P], BF16, kind="ExternalInput")
    iotacol_in = nc.dram_tensor("iotacol_in", [P, 1], BF16, kind="ExternalInput")
    ones_in = nc.dram_tensor("ones_in", [1, P], BF16, kind="ExternalInput")
    Wm, Mae, biasg = [], [], []
    for i in range(NL):
        Wm.append(nc.dram_tensor(f"W{i}", [DINS[i], D + 16], BF16,
                                 kind="ExternalInput"))
        Mae.append(nc.dram_tensor(f"Mae{i}", [ED, HS[i]], BF16, kind="ExternalInput"))
        biasg.append(nc.dram_tensor(f"bias{i}", [P, D], F32, kind="ExternalInput"))
    s2s_idx_in = nc.dram_tensor("s2s_idx_in", [P, TS], I32, kind="ExternalInput")
    s2s_mask_in = nc.dram_tensor("s2s_mask_in", [P, TS], F32, kind="ExternalInput")
    s2s_brel_in = nc.dram_tensor("s2s_brel_in", [P, TS], BF16, kind="ExternalInput")
    s2s_brelr_in = nc.dram_tensor("s2s_brelr_in", [1, TS * P], BF16,
                                  kind="ExternalInput")
    gfT_in = nc.dram_tensor("gfT_in", [GD, GPC], BF16, kind="ExternalInput")
    WihT = nc.dram_tensor("WihT", [2 * D, 4 * D], BF16, kind="ExternalInput")
    WhhT = nc.dram_tensor("WhhT", [D, 4 * D], BF16, kind="ExternalInput")
    s2s_bias = nc.dram_tensor("s2s_bias", [1, 4 * D], BF16, kind="ExternalInput")
    p1W = nc.dram_tensor("p1W", [2 * D + GD, D], BF16, kind="ExternalInput")
    p1b = nc.dram_tensor("p1b", [1, D], BF16, kind="ExternalInput")
    p2W = nc.dram_tensor("p2W", [D, D // 2], BF16, kind="ExternalInput")
    p2b = nc.dram_tensor("p2b", [1, D // 2], BF16, kind="ExternalInput")
    p3W = nc.dram_tensor("p3W", [D // 2, 5], BF16, kind="ExternalInput")
    p3b = nc.dram_tensor("p3b", [1, 5], BF16, kind="ExternalInput")
    out_t = nc.dram_tensor("out", [GPC, 5], F32, kind="ExternalOutput")
    if cfg.DBG:
        NR = cfg.NC * P * ((NPC + P - 1) // P)
        dbg_t = [nc.dram_tensor(f"dbg{li}", [NR, TWP], BF16,
                                kind="ExternalOutput") for li in range(NL)]
        dbgh_t = nc.dram_tensor("dbgh", [N, D], BF16, kind="ExternalOutput")
        dbghs_t = [nc.dram_tensor(f"dbghs{li}", [P, W * D], BF16,
                                  kind="ExternalOutput") for li in range(NL - 1)]
        dbght_t = [nc.dram_tensor(f"dbght{li}", [P, W * 2 * P], BF16,
                                  kind="ExternalOutput") for li in range(NL - 1)]
        KMAX = pack["KMAX"]
        dbgM_t = nc.dram_tensor("dbgM", [P, KMAX * P], BF16,
                                kind="ExternalOutput")
        dbgMT_t = nc.dram_tensor("dbgMT", [P, KMAX * P], BF16,
                                 kind="ExternalOutput")
        dbgex_t = nc.dram_tensor("dbgex", [P, KMAX * 8], BF16,
                                 kind="ExternalOutput")
        dbgal_t = nc.dram_tensor("dbgal", [P, KMAX * 8], BF16,
                                 kind="ExternalOutput")
        dbglg_t = nc.dram_tensor("dbglg", [P, KMAX * TWP], BF16,
                                 kind="ExternalOutput")

    T = dict(locals())
    with tile.TileContext(nc) as tc:
        for rep in range(reps):
            build_body(nc, tc, cfg, pack, T, sfx=f"r{rep}" if reps > 1 else "")
    nc.compile()
    return nc


def build_body(nc, tc, cfg, pack, T, sfx=""):
    N, NPC, W, GPC, TS = cfg.N, cfg.NPC, cfg.W, cfg.GPC, cfg.TS
    D, ED, GD = cfg.D, cfg.ED, cfg.GD
    TW = D + 8
    HS = [8, 8, 8, 1]
    DINS = [cfg.ND, D, D, D]
    NL = cfg.LAYERS
    K_w, KOFF, TK, KMAX = pack["K_w"], pack["KOFF"], pack["TK"], pack["KMAX"]
    RG = [list(range(cfg.NC))]
    SLAB = 4             # phase-A windows per store slab

    import contextlib
    ctx = contextlib.ExitStack()
    with ctx:
        pers = ctx.enter_context(tc.tile_pool(name="pers", bufs=1))
        dram = ctx.enter_context(tc.tile_pool(name="dram", bufs=1, space="DRAM"))

        # ---- persistent constants ----
        iota_sb = pers.tile([P, P], BF16, tag="iota")
        nc.sync.dma_start(iota_sb[:], T["iota_in"][:])
        iotac_sb = pers.tile([P, 1], BF16, tag="iotac")
        nc.sync.dma_start(iotac_sb[:], T["iotacol_in"][:])
        ones_sb = pers.tile([1, P], BF16, tag="ones")
        nc.sync.dma_start(ones_sb[:], T["ones_in"][:])
        ident_sb = pers.tile([P, P], F32, tag="ident")
        nc.vector.tensor_tensor(out=ident_sb[:],
                                in0=iotac_sb[:].to_broadcast([P, P]),
                                in1=iota_sb[:], op=OP.is_equal)
        ident16_sb = pers.tile([P, P], BF16, tag="ident16")
        nc.vector.tensor_copy(ident16_sb[:], ident_sb[:])
        # iotaRep[p, n, k] = n — packed-innermost counterpart of iota for the
        # indicator build (keeps every operand's last dim stride-1 so the DVE
        # runs in its 2x/4x mode)
        KMAX = pack["KMAX"]
        iotarep_sb = pers.tile([P, P, KMAX], BF16, tag="iotarep")
        nc.vector.tensor_copy(
            iotarep_sb[:],
            iota_sb[:, :, None].to_broadcast([P, P, KMAX]))

        # edge metadata, resident for the whole kernel
        idx16_all = pers.tile([P, 8 * TK], I16, tag="idx16_all")
        nc.sync.dma_start(idx16_all[:], T["idx16_in"][:])
        dcol_all = pers.tile([P, TK], BF16, tag="dcol_all")
        nc.sync.dma_start(dcol_all[:], T["dcol_in"][:])
        mask_all = pers.tile([P, TK], BF16, tag="mask_all")
        nc.sync.dma_start(mask_all[:], T["mask_in"][:])

        # transposed features ping-pong, resident in SBUF:
        # window w chunk c lives at cols [w*2P + c*P : w*2P + c*P + P)
        hT_sb = [pers.tile([P, W * 2 * P], BF16, tag=f"hT{pp}",
                           name=f"hT{pp}{sfx}")
                 for pp in range(2)]
        # residual h (bf16), resident: window w at cols [w*D:(w+1)*D)
        h_sb = pers.tile([P, W * D], BF16, tag="h_sb")

        # DRAM scratch (rows padded to TWP for 256B-aligned dma_gather);
        # table row order is [core][p][w] — see host src2row
        NR = cfg.NC * P * W
        lin_local = dram.tile([P, W * TWP], BF16, tag="lin_local")
        tables = [dram.tile([NR, TWP], BF16, tag=f"table{li}",
                            name=f"table{li}{sfx}",
                            addr_space="Shared") for li in range(NL)]
        hfin_local = dram.tile([NPC, D], BF16, tag="hfin_local")
        hfin_table = dram.tile([N, D], BF16, tag=f"hfin_table{sfx}",
                               addr_space="Shared")

        # ================= GAT layers =================
        with tc.tile_pool(name="lw", bufs=1) as lw, \
             tc.tile_pool(name="win", bufs=2) as win, \
             tc.tile_pool(name="psN", bufs=2, space="PSUM") as psN, \
             tc.tile_pool(name="psS", bufs=1, space="PSUM") as psS:
            for li in range(NL):
                H = HS[li]
                C = D // H
                din = DINS[li]
                nkc = (din + P - 1) // P   # contraction chunks for lin matmul

                # --- per-layer weights into SBUF ---
                DW = D + 16
                W_sb = lw.tile([P, nkc * DW], BF16, tag="W_sb")
                for c2 in range(nkc):
                    r0, r1 = c2 * P, min((c2 + 1) * P, din)
                    nc.sync.dma_start(W_sb[: r1 - r0, c2 * DW:(c2 + 1) * DW],
                                      T["Wm"][li][r0:r1, :])
                if pack["use_bias"]:
                    bias_b = lw.tile([P, D], F32, tag="bias_b")
                    nc.sync.dma_start(bias_b[:], T["biasg"][li][:])
                mae_sb = lw.tile([ED, 8], BF16, tag="mae_sb")
                nc.sync.dma_start(mae_sb[:, :H], T["Mae"][li][:])
                adst_all = lw.tile([P, W * 8], BF16, tag="adst_all")
                nc.vector.memset(adst_all[:], 0.0)

                hT_prev = hT_sb[li % 2]
                hT_next = hT_sb[(li + 1) % 2]

                # ---------- phase A: dense lin + a_src/a_dst ----------
                for s0 in range(0, W, SLAB):
                    s1 = min(s0 + SLAB, W)
                    lin16 = win.tile([P, SLAB, TWP], BF16, tag="lin16")
                    if li == 0:
                        rows0 = min(s1 * P, NPC) - s0 * P
                        xTw = win.tile([cfg.ND, SLAB * P], BF16, tag="xTw")
                        nc.sync.dma_start(xTw[:, :rows0],
                                          T["xT"][:, s0 * P:s0 * P + rows0])
                    for w in range(s0, s1):
                        n0 = w * P
                        cnt = min(P, NPC - n0)
                        j = w - s0
                        lin_ps = psS.tile([P, D + 16], F32, tag="scr_ps")
                        if li == 0:
                            nc.tensor.matmul(lin_ps[:cnt],
                                             lhsT=xTw[:, j * P:j * P + cnt],
                                             rhs=W_sb[:din, 0:DW],
                                             start=True, stop=True)
                        else:
                            for c2 in range(nkc):
                                nc.tensor.matmul(
                                    lin_ps[:cnt],
                                    lhsT=hT_prev[:, w * 2 * P + c2 * P:
                                                 w * 2 * P + c2 * P + cnt],
                                    rhs=W_sb[:, c2 * DW:(c2 + 1) * DW],
                                    start=(c2 == 0), stop=(c2 == nkc - 1))
                        # cols 0:D = lin (c-major), D:D+8 = a_src, D+8: = a_dst
                        nc.vector.tensor_copy(lin16[:cnt, j, 0:D + 8],
                                              lin_ps[:cnt, 0:D + 8])
                        nc.vector.tensor_copy(adst_all[:cnt, w * 8:w * 8 + 8],
                                              lin_ps[:cnt, D + 8:D + 16])
                    nj = s1 - s0
                    nc.sync.dma_start(
                        lin_local[:, s0 * TWP:s1 * TWP],
                        lin16[:, :nj, :].rearrange("p j c -> p (j c)"))

                # ---------- AllGather the transformed-feature table ----------
                if cfg.FAKE_AG:
                    nc.sync.dma_start(
                        tables[li][0:P * W, :].rearrange("(p w) c -> p w c",
                                                         p=P),
                        lin_local[:].rearrange("p (w c) -> p w c", w=W))
                else:
                    nc.gpsimd.collective_compute(
                        "AllGather", OP.bypass, replica_groups=RG,
                        ins=[lin_local[:]], outs=[tables[li][:]])

                # ---------- phase B: per-window edge aggregation ----------
                qrot = [0]
                for w in range(W):
                    n0 = w * P
                    cnt = min(P, NPC - n0)
                    K = K_w[w]
                    o0 = int(KOFF[w])
                    seaT_sb = win.tile([ED, KMAX * P], BF16, tag="seaT_sb")
                    nc.sync.dma_start(seaT_sb[:, :K * P],
                                      T["srow_in"][:, o0 * P:(o0 + K) * P])

                    TA = pack["TA_w"][w]
                    TB = pack["TB_w"][w]
                    SPLIT = min(cfg.SPLIT, NR)
                    lin_g = win.tile([P, KMAX, TWP], BF16, tag="lin_g")
                    for tbase, tcnt, r0, r1 in ((0, TA, 0, SPLIT),
                                                (TA, TB, SPLIT, NR)):
                        for g0 in range(0, tcnt, 7):
                            g1 = min(g0 + 7, tcnt)
                            t0, t1 = tbase + g0, tbase + g1
                            nc.gpsimd.dma_gather(
                                out_ap=lin_g[:, t0:t1, :],
                                in_ap=tables[li][r0:r1, :],
                                idxs_ap=idx16_all[:, 8 * (o0 + t0):
                                                  8 * (o0 + t1)],
                                num_idxs=(t1 - t0) * P,
                                num_idxs_reg=(t1 - t0) * P,
                                elem_size=TWP, queue_num=qrot[0] % 4)
                            qrot[0] += 1

                    # indicator M [edge_p, node, k] (k innermost => DVE 2x);
                    # MT = M^T via PE transpose + scalar-engine PSUM copy
                    M_sb = win.tile([P, P, KMAX], BF16, tag="M_sb")
                    nc.vector.tensor_tensor(
                        out=M_sb[:, :, :K],
                        in0=dcol_all[:, None, o0:o0 + K].to_broadcast(
                            [P, P, K]),
                        in1=iotarep_sb[:, :, :K],
                        op=OP.is_equal)
                    MT_sb = win.tile([P, KMAX * P], BF16, tag="MT_sb")
                    trM_ps = psS.tile([P, KMAX * P], BF16, tag="trM_ps")
                    for k in range(K):
                        nc.tensor.transpose(trM_ps[:, k * P:(k + 1) * P],
                                            M_sb[:, :, k], ident16_sb[:])
                    nc.scalar.activation(MT_sb[:, :K * P], trM_ps[:, :K * P],
                                         AF.Copy)

                    # alpha = lrelu(a_src + a_dst + a_edge); ex = exp * mask
                    # (al shares the PSUM bank with nu at columns 264+)
                    acc_ps = psN.tile([P, D + 8 + KMAX * 8], F32, tag="acc_ps")
                    AL0 = D + 8
                    for k in range(K):
                        nc.tensor.matmul(acc_ps[:, AL0 + k * 8:AL0 + k * 8 + H],
                                         lhsT=MT_sb[:, k * P:(k + 1) * P],
                                         rhs=adst_all[:, w * 8:w * 8 + H],
                                         start=True, stop=False)
                        nc.tensor.matmul(acc_ps[:, AL0 + k * 8:AL0 + k * 8 + H],
                                         lhsT=seaT_sb[:, k * P:(k + 1) * P],
                                         rhs=mae_sb[:, :H],
                                         start=False, stop=True)
                    al_sb = win.tile([P, KMAX * 8], BF16, tag="al_sb")
                    nc.vector.tensor_tensor(
                        out=al_sb[:, :K * 8].rearrange(
                            "p (k h) -> p k h", k=K)[:, :, :H],
                        in0=acc_ps[:, AL0:AL0 + K * 8].rearrange(
                            "p (k h) -> p k h", k=K)[:, :, :H],
                        in1=lin_g[:, :K, D:D + H], op=OP.add)
                    # leaky_relu(x, 0.2) = max(0.2*x, x)
                    lr_sb = win.tile([P, KMAX * 8], BF16, tag="lr_sb")
                    nc.scalar.activation(lr_sb[:, :K * 8], al_sb[:, :K * 8],
                                         AF.Lrelu, alpha=0.2)
                    ex_sb = win.tile([P, KMAX * 8], BF16, tag="ex_sb")
                    nc.scalar.activation(ex_sb[:, :K * 8], lr_sb[:, :K * 8],
                                         AF.Exp)
                    # mask pad slots; for H=1 also duplicate the per-edge
                    # weight across the 8 lanes so the wfex multiply can use
                    # the packed-h fast path
                    nc.vector.tensor_tensor(
                        out=ex_sb[:, :K * 8].rearrange("p (k h) -> p k h", k=K),
                        in0=(ex_sb[:, :K * 8].rearrange("p (k h) -> p k h",
                                                        k=K)
                             if H > 1 else
                             ex_sb[:, :K * 8].rearrange("p (k h) -> p k h",
                                                        k=K)[:, :, 0:1
                                 ].to_broadcast([P, K, 8])),
                        in1=mask_all[:, o0:o0 + K][:, :, None].to_broadcast(
                            [P, K, 8]),
                        op=OP.mult)

                    if cfg.DBG and li == 0 and w == 0:
                        nc.sync.dma_start(T["dbgM_t"][:, :K * P],
                                          M_sb[:, :K, :].rearrange(
                                              "p k n -> p (k n)"))
                        nc.sync.dma_start(T["dbgMT_t"][:, :K * P],
                                          MT_sb[:, :K * P])
                        nc.sync.dma_start(T["dbgex_t"][:, :K * 8],
                                          ex_sb[:, :K * 8])
                        nc.sync.dma_start(T["dbgal_t"][:, :K * 8],
                                          al_sb[:, :K * 8])
                        nc.sync.dma_start(T["dbglg_t"][:, :K * TWP],
                                          lin_g[:, :K, :].rearrange(
                                              "p k c -> p (k c)"))

                    # weighted features + segment sums (low half of acc_ps)
                    for b0 in range(0, K, 4):
                        b1 = min(b0 + 4, K)
                        nb = b1 - b0
                        wfex = win.tile([P, 4, D + 8], BF16, tag="wfex")
                        # c-major feature layout: the 8-lane dim is the packed
                        # innermost dim for all three operands (for H=1 the
                        # lanes hold 8 copies of the per-edge weight)
                        nc.vector.tensor_tensor(
                            out=wfex[:, :nb, 0:D].rearrange(
                                "p k (c h) -> p k c h", h=8),
                            in0=lin_g[:, b0:b1, 0:D].rearrange(
                                "p k (c h) -> p k c h", h=8),
                            in1=ex_sb[:, b0 * 8:b1 * 8].rearrange(
                                "p (k h) -> p k h", k=nb)[:, :, None, :
                                ].to_broadcast([P, nb, D // 8, 8]),
                            op=OP.mult)
                        nc.vector.tensor_copy(
                            wfex[:, :nb, D:D + 8],
                            ex_sb[:, b0 * 8:b1 * 8].rearrange(
                                "p (k h) -> p k h", k=nb))
                        for kk in range(nb):
                            k = b0 + kk
                            nc.tensor.matmul(
                                acc_ps[:, 0:D + H],
                                lhsT=M_sb[:, :, k],
                                rhs=wfex[:, kk, 0:D + H],
                                start=(k == 0), stop=(k == K - 1))

                    # normalize, bias, ELU, residual (bf16 epilogue); den > 0
                    # is guaranteed by the self-loop edge, so no epsilon
                    rec = win.tile([P, 8], F32, tag="rec")
                    nc.vector.reciprocal(rec[:cnt, :H], acc_ps[:cnt, D:D + H])
                    outw = win.tile([P, D], BF16, tag="outw")
                    nc.vector.tensor_tensor(
                        out=outw[:cnt].rearrange("p (c h) -> p c h", h=H),
                        in0=acc_ps[:cnt, 0:D].rearrange("p (c h) -> p c h",
                                                        h=H),
                        in1=rec[:cnt, None, :H].to_broadcast(
                            [cnt, D // H, H]),
                        op=OP.mult)
                    if pack["use_bias"]:
                        nc.vector.tensor_tensor(out=outw[:cnt], in0=outw[:cnt],
                                                in1=bias_b[:cnt], op=OP.add)
                    # ELU = relu(x) + exp(min(x,0)) - 1
                    tmin = win.tile([P, D], BF16, tag="tmin")
                    nc.vector.tensor_scalar_min(tmin[:cnt], outw[:cnt], 0.0)
                    nc.scalar.activation(tmin[:cnt], tmin[:cnt], AF.Exp)
                    nc.vector.tensor_scalar_max(outw[:cnt], outw[:cnt], 0.0)
                    hn = win.tile([P, D], BF16, tag="hn")
                    if cnt < P:
                        nc.vector.memset(hn[:], 0.0)
                    nc.vector.tensor_tensor(out=hn[:cnt], in0=tmin[:cnt],
                                            in1=outw[:cnt], op=OP.add)
                    nc.vector.tensor_scalar_add(hn[:cnt], hn[:cnt], -1.0)
                    if li > 0:
                        nc.vector.tensor_tensor(out=hn[:cnt], in0=hn[:cnt],
                                                in1=h_sb[:cnt, w * D:w * D + D],
                                                op=OP.add)
                    if li < NL - 1:
                        nc.vector.tensor_copy(h_sb[:, w * D:(w + 1) * D], hn[:])
                        tr16_ps = psS.tile([P, D], BF16, tag="tr16_ps")
                        for c2 in range(2):
                            nc.tensor.transpose(tr16_ps[:, c2 * P:(c2 + 1) * P],
                                                hn[:, c2 * P:(c2 + 1) * P],
                                                ident16_sb[:])
                            nc.vector.tensor_copy(
                                hT_next[:, w * 2 * P + c2 * P:
                                        w * 2 * P + c2 * P + cnt],
                                tr16_ps[:, c2 * P:c2 * P + cnt])
                    else:
                        nc.sync.dma_start(hfin_local[n0:n0 + cnt], hn[:cnt])

                if cfg.DBG and li < NL - 1:
                    nc.sync.dma_start(T["dbghs_t"][li][:], h_sb[:])
                    nc.sync.dma_start(T["dbght_t"][li][:], hT_next[:])

            # final AllGather of node features for set2set
            if cfg.FAKE_AG:
                nc.sync.dma_start(hfin_table[0:NPC, :], hfin_local[:])
            else:
                nc.gpsimd.collective_compute(
                    "AllGather", OP.bypass, replica_groups=RG,
                    ins=[hfin_local[:]], outs=[hfin_table[:]])
            if cfg.DBG:
                for li in range(NL):
                    nc.sync.dma_start(T["dbg_t"][li][:], tables[li][:])
                nc.sync.dma_start(T["dbgh_t"][:], hfin_table[:])

        # ================= Set2Set + MLP head =================
        build_s2s(nc, tc, cfg, T, pers, dram, hfin_table,
                  iota_sb, iotac_sb, ones_sb, ident_sb, sfx=sfx)


def build_s2s(nc, tc, cfg, T, pers, dram, hfin_table,
              iota_sb, iotac_sb, ones_sb, ident_sb, sfx=""):
    N, NPC, GPC, TS = cfg.N, cfg.NPC, cfg.GPC, cfg.TS
    D, GD = cfg.D, cfg.GD
    GG = GPC
    STEPS = cfg.S2S_STEPS

    with tc.tile_pool(name="s2s", bufs=1) as sp, \
         tc.tile_pool(name="ps2", bufs=1, space="PSUM") as ps2:
        # gather this core's node features (padded to TS*128)
        s2s_idx = sp.tile([P, TS], I32, tag="s2s_idx")
        nc.sync.dma_start(s2s_idx[:], T["s2s_idx_in"][:])
        xn = sp.tile([P, TS, D], BF16, tag="xn")
        for t in range(TS):
            nc.gpsimd.indirect_dma_start(
                out=xn[:, t, :], out_offset=None, in_=hfin_table[:],
                in_offset=bass.IndirectOffsetOnAxis(
                    ap=s2s_idx[:, t:t + 1], axis=0))
        maskc = sp.tile([P, TS], F32, tag="maskc")
        nc.sync.dma_start(maskc[:], T["s2s_mask_in"][:])
        brelc = sp.tile([P, TS], BF16, tag="brelc")
        nc.sync.dma_start(brelc[:], T["s2s_brel_in"][:])
        brelr = sp.tile([1, TS * P], BF16, tag="brelr")
        nc.sync.dma_start(brelr[:], T["s2s_brelr_in"][:])

        # indicator matrices per node tile (once for all steps)
        Mb = sp.tile([P, TS * GG], BF16, tag="Mb")       # node_p x graph_f
        nc.vector.tensor_tensor(
            out=Mb[:].rearrange("p (t g) -> p t g", t=TS),
            in0=brelc[:, :, None].to_broadcast([P, TS, GG]),
            in1=iota_sb[:, None, 0:GG].to_broadcast([P, TS, GG]),
            op=OP.is_equal)
        MbT = sp.tile([GG, TS * P], BF16, tag="MbT")     # graph_p x node_f
        for t in range(TS):
            bc_ps = ps2.tile([GG, P], F32, tag="psX")
            nc.tensor.matmul(bc_ps[:], lhsT=ones_sb[:, 0:GG],
                             rhs=brelr[:, t * P:(t + 1) * P], start=True,
                             stop=True)
            nc.vector.tensor_tensor(
                out=MbT[:, t * P:(t + 1) * P],
                in0=iotac_sb[:GG].to_broadcast([GG, P]),
                in1=bc_ps[:], op=OP.is_equal)

        # s2s weights: WihT [2D,4D] in 4 row-chunks, WhhT [D,4D] in 2 chunks
        wih = sp.tile([P, 4 * 4 * D], BF16, tag="wih")
        for c2 in range(4):
            nc.sync.dma_start(wih[:, c2 * 4 * D:(c2 + 1) * 4 * D],
                              T["WihT"][c2 * P:(c2 + 1) * P, :])
        whh = sp.tile([P, 2 * 4 * D], BF16, tag="whh")
        for c2 in range(2):
            nc.sync.dma_start(whh[:, c2 * 4 * D:(c2 + 1) * 4 * D],
                              T["WhhT"][c2 * P:(c2 + 1) * P, :])
        s2sb = sp.tile([1, 4 * D], BF16, tag="s2sb")
        nc.sync.dma_start(s2sb[:], T["s2s_bias"][:])

        # LSTM / attention state: q*^T chunks (h part then r part), h^T, c
        qT = [sp.tile([P, GG], BF16, tag=f"qT{c2}", name=f"qT{c2}{sfx}")
              for c2 in range(4)]
        c_st = sp.tile([GG, D], F32, tag="c_st")
        for t_ in qT:
            nc.vector.memset(t_[:], 0.0)
        nc.vector.memset(c_st[:], 0.0)

        gact = [AF.Sigmoid, AF.Sigmoid, AF.Tanh, AF.Sigmoid]  # i, f, g, o
        for step in range(STEPS):
            gs = []
            for g in range(4):
                g_ps = ps2.tile([GG, D], F32, tag="psY")
                nc.tensor.matmul(g_ps[:], lhsT=ones_sb[:, 0:GG],
                                 rhs=s2sb[:, g * D:(g + 1) * D],
                                 start=True, stop=False)
                for c2 in range(4):
                    nc.tensor.matmul(
                        g_ps[:], lhsT=qT[c2][:],
                        rhs=wih[:, c2 * 4 * D + g * D: c2 * 4 * D + (g + 1) * D],
                        start=False, stop=False)
                for c2 in range(2):
                    # h part of q_star doubles as the LSTM h for Whh
                    nc.tensor.matmul(
                        g_ps[:], lhsT=qT[c2][:],
                        rhs=whh[:, c2 * 4 * D + g * D: c2 * 4 * D + (g + 1) * D],
                        start=False, stop=(c2 == 1))
                g_sb = sp.tile([GG, D], F32, tag=f"g_sb{g}")
                nc.scalar.activation(g_sb[:], g_ps[:], gact[g])
                gs.append(g_sb)
            # c = f*c + i*tanh(g);  h = o*tanh(c)
            t1 = sp.tile([GG, D], F32, tag="t1")
            nc.vector.tensor_tensor(out=t1[:], in0=gs[0][:], in1=gs[2][:],
                                    op=OP.mult)
            nc.vector.tensor_tensor(out=c_st[:], in0=gs[1][:], in1=c_st[:],
                                    op=OP.mult)
            nc.vector.tensor_tensor(out=c_st[:], in0=c_st[:], in1=t1[:],
                                    op=OP.add)
            tc_sb = sp.tile([GG, D], F32, tag="tc_sb")
            nc.scalar.activation(tc_sb[:], c_st[:], AF.Tanh)
            h_l = sp.tile([GG, D], F32, tag="h_l")
            nc.vector.tensor_tensor(out=h_l[:], in0=gs[3][:], in1=tc_sb[:],
                                    op=OP.mult)
            h16 = sp.tile([GG, D], BF16, tag="h16")
            nc.vector.tensor_copy(h16[:], h_l[:])

            # attention over nodes: e = <xn, h[batch]>, softmax per graph
            e_all = sp.tile([P, TS], F32, tag="e_all")
            escr = sp.tile([P, D], F32, tag="escr")
            for t in range(TS):
                he_ps = ps2.tile([P, D], F32, tag="psH")
                nc.tensor.matmul(he_ps[:], lhsT=MbT[:, t * P:(t + 1) * P],
                                 rhs=h16[:], start=True, stop=True)
                nc.vector.tensor_tensor(out=escr[:], in0=xn[:, t, :],
                                        in1=he_ps[:], op=OP.mult)
                nc.vector.reduce_sum(out=e_all[:, t:t + 1], in_=escr[:],
                                     axis=mybir.AxisListType.X)
            nc.scalar.activation(e_all[:], e_all[:], AF.Exp)
            nc.vector.tensor_tensor(out=e_all[:], in0=e_all[:], in1=maskc[:],
                                    op=OP.mult)
            e16 = sp.tile([P, TS], BF16, tag="e16")
            nc.vector.tensor_copy(e16[:], e_all[:])
            r_ps = ps2.tile([GG, D + 1], F32, tag="psR")
            for t in range(TS):
                wxex = sp.tile([P, D + 1], BF16, tag="wxex")
                nc.vector.tensor_tensor(
                    out=wxex[:, 0:D], in0=xn[:, t, :],
                    in1=e16[:, t:t + 1].to_broadcast([P, D]), op=OP.mult)
                nc.vector.tensor_copy(wxex[:, D:D + 1], e16[:, t:t + 1])
                nc.tensor.matmul(r_ps[:], lhsT=Mb[:, t * GG:(t + 1) * GG],
                                 rhs=wxex[:], start=(t == 0), stop=(t == TS - 1))
            den = sp.tile([GG, 1], F32, tag="s2s_den")
            nc.vector.tensor_scalar_add(den[:], r_ps[:, D:D + 1], 1e-16)
            rec = sp.tile([GG, 1], F32, tag="s2s_rec")
            nc.vector.reciprocal(rec[:], den[:])
            r_sb = sp.tile([GG, D], F32, tag="r_sb")
            nc.vector.tensor_tensor(out=r_sb[:], in0=r_ps[:, 0:D],
                                    in1=rec[:].to_broadcast([GG, D]),
                                    op=OP.mult)
            # q_star^T = [h^T | r^T] for next step / head
            for c2 in range(2):
                tr_ps = ps2.tile([P, GG], F32, tag="psX")
                nc.tensor.transpose(tr_ps[:], h_l[:, c2 * P:(c2 + 1) * P],
                                    ident_sb[:GG, :GG])
                nc.vector.tensor_copy(qT[c2][:], tr_ps[:])
                tr_ps2 = ps2.tile([P, GG], F32, tag="psX")
                nc.tensor.transpose(tr_ps2[:], r_sb[:, c2 * P:(c2 + 1) * P],
                                    ident_sb[:GG, :GG])
                nc.vector.tensor_copy(qT[2 + c2][:], tr_ps2[:])

        # ---------------- MLP head ----------------
        gfT_sb = sp.tile([GD, GG], BF16, tag="gfT_sb")
        nc.sync.dma_start(gfT_sb[:], T["gfT_in"][:])
        p1w_sb = sp.tile([P, 4 * D], BF16, tag="p1w_sb")
        for c2 in range(4):
            nc.sync.dma_start(p1w_sb[:, c2 * D:(c2 + 1) * D],
                              T["p1W"][c2 * P:(c2 + 1) * P, :])
        p1wg_sb = sp.tile([GD, D], BF16, tag="p1wg_sb")
        nc.sync.dma_start(p1wg_sb[:], T["p1W"][4 * P:4 * P + GD, :])
        p1b_sb = sp.tile([1, D], BF16, tag="p1b_sb")
        nc.sync.dma_start(p1b_sb[:], T["p1b"][:])
        z1_ps = ps2.tile([GG, D], F32, tag="psY")
        nc.tensor.matmul(z1_ps[:], lhsT=ones_sb[:, 0:GG], rhs=p1b_sb[:],
                         start=True, stop=False)
        for c2 in range(4):
            nc.tensor.matmul(z1_ps[:], lhsT=qT[c2][:],
                             rhs=p1w_sb[:, c2 * D:(c2 + 1) * D],
                             start=False, stop=False)
        nc.tensor.matmul(z1_ps[:], lhsT=gfT_sb[:], rhs=p1wg_sb[:],
                         start=False, stop=True)
        z1 = sp.tile([GG, D], F32, tag="z1")
        nc.scalar.activation(z1[:], z1_ps[:], AF.Relu)

        p2w_sb = sp.tile([P, 2 * (D // 2)], BF16, tag="p2w_sb")
        for c2 in range(2):
            nc.sync.dma_start(p2w_sb[:, c2 * (D // 2):(c2 + 1) * (D // 2)],
                              T["p2W"][c2 * P:(c2 + 1) * P, :])
        p2b_sb = sp.tile([1, D // 2], BF16, tag="p2b_sb")
        nc.sync.dma_start(p2b_sb[:], T["p2b"][:])
        z2_ps = ps2.tile([GG, D // 2], F32, tag="psY")
        nc.tensor.matmul(z2_ps[:], lhsT=ones_sb[:, 0:GG], rhs=p2b_sb[:],
                         start=True, stop=False)
        for c2 in range(2):
            z1T_ps = ps2.tile([P, GG], F32, tag="psX")
            nc.tensor.transpose(z1T_ps[:], z1[:, c2 * P:(c2 + 1) * P],
                                ident_sb[:GG, :GG])
            z1T = sp.tile([P, GG], BF16, tag="z1T")
            nc.vector.tensor_copy(z1T[:], z1T_ps[:])
            nc.tensor.matmul(z2_ps[:], lhsT=z1T[:],
                             rhs=p2w_sb[:, c2 * (D // 2):(c2 + 1) * (D // 2)],
                             start=False, stop=(c2 == 1))
        z2 = sp.tile([GG, D // 2], F32, tag="z2")
        nc.scalar.activation(z2[:], z2_ps[:], AF.Relu)

        p3w_sb = sp.tile([D // 2, 5], BF16, tag="p3w_sb")
        nc.sync.dma_start(p3w_sb[:], T["p3W"][:])
        p3b_sb = sp.tile([1, 5], BF16, tag="p3b_sb")
        nc.sync.dma_start(p3b_sb[:], T["p3b"][:])
        z2T_ps = ps2.tile([P, GG], F32, tag="psX")
        nc.tensor.transpose(z2T_ps[:], z2[:], ident_sb[:GG, :GG])
        z2T = sp.tile([P, GG], BF16, tag="z2T")
        nc.vector.tensor_copy(z2T[:], z2T_ps[:])
        o_ps = ps2.tile([GG, 5], F32, tag="psY")
        nc.tensor.matmul(o_ps[:], lhsT=ones_sb[:, 0:GG], rhs=p3b_sb[:],
                         start=True, stop=False)
        nc.tensor.matmul(o_ps[:], lhsT=z2T[:], rhs=p3w_sb[:],
                         start=False, stop=True)
        o_sb = sp.tile([GG, 5], F32, tag="o_sb")
        nc.vector.tensor_copy(o_sb[:], o_ps[:])
        nc.sync.dma_start(T["out_t"][:], o_sb[:cfg.GPC])


def run_config(inputs, cfg):
    in_maps, pack = host_prep(inputs, cfg)
    nc = build_kernel(cfg, pack)
    res = run_bass_kernel_spmd(nc, in_maps, core_ids=list(range(cfg.NC)))
    out = np.concatenate([res.results[c]["out"] for c in range(cfg.NC)], axis=0)
    return out.astype(np.float32)


def kernel(**inputs):
    return run_config(inputs, CFG.derive())


# revision 97
# speedup vs baseline: 1.9094x; 1.0327x over previous
"""Trainium2 Bass kernel for AdvancedGATModel (4-layer edge-featured GAT +
Set2Set pooling + MLP head), sharded across 8 NeuronCores.

Sharding: nodes are split into 8 contiguous slices (6250 each); each core owns
the edges whose *destination* lands in its slice (plus self-loops), so segment
softmax and the scatter-add aggregation are core-local.  Per layer each core
computes the linear transform of its node slice, all cores AllGather the
transformed features (bf16) into a replicated table, and each core gathers its
edges' source rows via batched indirect DMA.  Segment softmax/weighted-sum run
as dense 128-edge-tile bf16 matmuls against on-the-fly one-hot "indicator"
matrices (edges x window-nodes).  Set2Set is sharded by graph (64 graphs/core;
graphs never straddle cores), and the [64,5] head outputs are concatenated on
host.

Perf notes vs the original version:
 - everything feeding the tensor engine is bf16 (4x column rate, fp32 PSUM
   accumulate), halves gather/AllGather bytes;
 - indirect gathers are batched (<=7 edge-tiles per SWDGE instruction instead
   of 1) to amortize the ~1us fixed SWDGE cost;
 - per-edge metadata (idx/dcol/mask) is loaded into SBUF once for the whole
   kernel; drow+edge-attrs are one DMA per window;
 - hT (transposed features) and the residual h live in SBUF across layers, so
   phase A reads weights straight from SBUF and phase B's residual add needs
   no DMA;
 - per-window edge-tile count K_w is the max over cores of what's needed
   (instead of a global worst case).

The program is identical on all 8 cores (SPMD); only input *data* differs.
All shapes below are hardcoded for the grading problem.
"""

import numpy as np
import ml_dtypes

import concourse.bass as bass
import concourse.bacc as bacc
import concourse.tile as tile
import concourse.mybir as mybir
from concourse.bass_utils import run_bass_kernel_spmd

F32 = mybir.dt.float32
BF16 = mybir.dt.bfloat16
I32 = mybir.dt.int32
I16 = mybir.dt.int16
AF = mybir.ActivationFunctionType
OP = mybir.AluOpType
P = 128
BF = ml_dtypes.bfloat16
TWP = 384       # gather-table row, padded to 256B-multiples for dma_gather


class CFG:
    # full problem; small-mode tests override these
    N = 50000          # nodes
    E = 800000         # edges (before self loops)
    G = 512            # graphs
    ND = 14            # node feat dim
    ED = 4             # edge feat dim
    GD = 13            # global feat dim
    D = 256            # hidden
    H = 8              # heads
    C = 32             # per-head channels
    NC = 8             # cores
    NPC = N // NC      # nodes per core = 6250
    W = 49             # node windows per core (ceil(NPC/128))
    GPC = G // NC      # graphs per core = 64
    TS = 51            # set2set node tiles per core (capacity TS*128 nodes)
    S2S_STEPS = 3
    LAYERS = 4
    FAKE_AG = False     # replace AllGathers with local copies (debug only)
    DBG = False         # dump per-layer tables as extra outputs (debug only)

    @classmethod
    def derive(cls):
        cls.NPC = cls.N // cls.NC
        cls.GPC = cls.G // cls.NC
        cls.W = -(-cls.NPC // P)
        return cls


def make_small_cfg():
    class Small(CFG):
        N = 2048
        E = 8192
        G = 64
        TS = 3
    return Small.derive()


# ------------------------------------------------------------------
# host-side preprocessing
# ------------------------------------------------------------------

def host_prep(inp, cfg):
    """Build per-core input maps from the full input dict.

    Returns (in_maps, pack) where pack carries the SPMD-uniform edge packing
    (per-window tile counts) that build_kernel hardcodes into the program.
    """
    N, E, G = cfg.N, cfg.E, cfg.G
    NC, NPC, W, GPC, TS = cfg.NC, cfg.NPC, cfg.W, cfg.GPC, cfg.TS
    D, H, C, ED, GD = cfg.D, cfg.H, cfg.C, cfg.ED, cfg.GD

    src = np.asarray(inp["edge_index"][0])
    dst = np.asarray(inp["edge_index"][1])
    ea = np.asarray(inp["edge_attr"], dtype=np.float32)
    batch = np.asarray(inp["batch_idx"])
    x = np.asarray(inp["x"], dtype=np.float32)

    # self-loop attr = mean incoming edge attr (0 for isolated nodes)
    deg = np.bincount(dst, minlength=N).astype(np.float32)
    loop = np.zeros((N, ED), np.float32)
    for j in range(ED):
        loop[:, j] = np.bincount(dst, weights=ea[:, j], minlength=N)
    loop /= np.maximum(deg, 1.0)[:, None]

    src2 = np.concatenate([src, np.arange(N, dtype=np.int64)])
    dst2 = np.concatenate([dst, np.arange(N, dtype=np.int64)])
    ea2 = np.concatenate([ea, loop], axis=0).astype(np.float32)

    order = np.argsort(dst2, kind="stable")
    s_src = src2[order]
    s_dst = dst2[order]
    s_ea = ea2[order]

    # The gather table is split into NCH window-range chunks, each its own
    # Shared tensor (one AllGather writer each, so chunks can fire early and
    # overlap compute).  Chunk cb covers windows [WB[cb], WB[cb+1]); within a
    # chunk, rows are [core][p][w] so phase A stores stay contiguous:
    #   src node s (c=s//NPC, w=(s%NPC)//P, p=s%P, chunk cb(w)) ->
    #   chunk-row  c*P*(WB[cb+1]-WB[cb]) + p*(WB[cb+1]-WB[cb]) + (w-WB[cb])
    # Each chunk has NC*P*17 <= 17408 rows, within dma_gather's int16 range.
    NCH = 3 if W > 6 else min(2, W)
    WB = [round(cb * W / NCH) for cb in range(NCH + 1)]
    wchunk = np.zeros(W, np.int64)
    for cb in range(NCH):
        wchunk[WB[cb]:WB[cb + 1]] = cb

    WBa = np.asarray(WB)

    def src2chunkrow(s):
        c, m = s // NPC, s % NPC
        p, w = m % P, m // P
        cb = wchunk[w]
        wr = WBa[cb + 1] - WBa[cb]
        return cb, c * P * wr + p * wr + (w - WBa[cb])

    core_groups = []
    cntG = np.zeros((NC, W, NCH), np.int64)
    for cidx in range(NC):
        n0, n1 = cidx * NPC, (cidx + 1) * NPC
        e0, e1 = np.searchsorted(s_dst, n0), np.searchsorted(s_dst, n1)
        cs, cd, cea = s_src[e0:e1], s_dst[e0:e1] - n0, s_ea[e0:e1]
        per_w = []
        for w in range(W):
            lo, hi = w * P, min((w + 1) * P, NPC)
            a = np.searchsorted(cd, lo)
            b2 = np.searchsorted(cd, hi)
            sl = slice(a, b2)
            gcb, grow = src2chunkrow(cs[sl])
            per_w.append((gcb, grow, (cd[sl] - lo).astype(np.float32),
                          cea[sl]))
            for cb in range(NCH):
                cntG[cidx, w, cb] = (gcb == cb).sum()
        core_groups.append(per_w)
    # per-window, per-chunk tile counts (max over cores, SPMD-uniform)
    TG_w = [[int(-(-cntG[:, w, cb].max() // P)) for cb in range(NCH)]
            for w in range(W)]
    for w in range(W):
        if sum(TG_w[w]) == 0:
            TG_w[w][0] = 1   # keep one (pad) gather so lin_g stays finite
    K_w = [sum(TG_w[w]) for w in range(W)]
    KOFF = np.concatenate([[0], np.cumsum(K_w)]).astype(np.int64)
    TK = int(KOFF[-1])
    KMAX = max(K_w)
    pack = dict(K_w=K_w, TG_w=TG_w, KOFF=KOFF, TK=TK, KMAX=KMAX,
                NCH=NCH, WB=WB)
    # filled below once weights are inspected
    pack["use_bias"] = False

    # per-layer host-packed weight helpers.
    # Features are stored CHANNEL-MAJOR on device: new col (c,h) = c*H8 + h
    # with H8=8 (layer 3's H=1 output adopts the same permutation so the
    # residual add stays elementwise-consistent).  All weight matrices are
    # permuted here on the host; a_src/a_dst are folded into the lin matmul
    # as two extra 8-wide column blocks Wa = W @ asrcBD, Wd = W @ adstBD.
    PERM = (np.arange(8)[None, :] * (D // 8)
            + np.arange(D // 8)[:, None]).flatten()          # [D] new->old

    def pack_layer(i):
        Wm = np.asarray(inp[f"g{i}_W"], np.float32)          # [din, H*C]
        We = np.asarray(inp[f"g{i}_We"], np.float32)         # [ED, H*C]
        asrc = np.asarray(inp[f"g{i}_asrc"], np.float32)     # [h, c]
        adst = np.asarray(inp[f"g{i}_adst"], np.float32)
        aedge = np.asarray(inp[f"g{i}_aedge"], np.float32)
        b = np.asarray(inp[f"g{i}_b"], np.float32)
        h, c = asrc.shape
        hc = h * c
        # M_ae[d, h] = sum_c We[d, h*c+cc] * aedge[h, cc]
        M_ae = (We.reshape(ED, h, c) * aedge[None]).sum(-1)  # [ED, h]
        # a_src[n, :] = h_in[n] @ Wa  (block-diag contraction over c)
        asrcBD = np.zeros((hc, 8), np.float32)
        adstBD = np.zeros((hc, 8), np.float32)
        for hh in range(h):
            asrcBD[hh * c:(hh + 1) * c, hh] = asrc[hh]
            adstBD[hh * c:(hh + 1) * c, hh] = adst[hh]
        Wa = Wm @ asrcBD                                     # [din, 8]
        Wd = Wm @ adstBD
        Wp = Wm[:, PERM]                                     # output c-major
        bp = b[PERM]
        if i > 0:                                            # input also perm
            Wp = Wp[PERM]
            Wa = Wa[PERM]
            Wd = Wd[PERM]
        Wfull = np.concatenate([Wp, Wa, Wd], axis=1)         # [din, D+16]
        return dict(W=Wfull, M_ae=M_ae, b=bp.reshape(1, hc), H=h, C=c)

    layers = [pack_layer(i) for i in range(4)]
    pack["use_bias"] = bool(any(np.abs(np.asarray(inp[f"g{i}_b"])).max() > 0
                                for i in range(4)))

    # graph ranges per core for set2set (graphs never straddle cores)
    gbound = np.searchsorted(batch, np.arange(G + 1))  # node start of each graph

    in_maps = []
    for cidx in range(NC):
        n0, n1 = cidx * NPC, (cidx + 1) * NPC

        # flat edge packing: window w owns tiles KOFF[w]..KOFF[w+1]; within a
        # window, chunk cb's edges fill tiles [sum(TG[:cb]), sum(TG[:cb+1]))
        idx16 = np.zeros((P, 8 * TK), np.int16)    # [16-band replicated x8]
        dcol_fl = np.zeros((P, TK), BF)            # dst rel to window (0..127)
        mask_fl = np.zeros((P, TK), BF)
        seaT_fl = np.zeros((ED, TK * P), BF)       # edge attrs, transposed
        for w in range(W):
            gcb, grow, rel_w, cea = core_groups[cidx][w]
            t0 = 0
            for cb in range(NCH):
                sel = gcb == cb
                es = grow[sel]
                rel = rel_w[sel]
                ea = cea[sel]
                cnt = len(es)
                if cnt:
                    js = np.arange(cnt)
                    tk = KOFF[w] + t0 + js // P
                    pp = js % P
                    dcol_fl[pp, tk] = rel.astype(BF)
                    mask_fl[pp, tk] = 1.0
                    seaT_fl[:, tk * P + pp] = ea.T.astype(BF)
                    cols = 8 * (KOFF[w] + t0) + js // 16
                    idx16[js % 16, cols] = es.astype(np.int16)
                t0 += TG_w[w][cb]
        for band in range(1, 8):
            idx16[16 * band:16 * (band + 1)] = idx16[:16]

        # set2set: node range + padding for this core's graphs
        g0 = cidx * GPC
        gn0, gn1 = gbound[g0], gbound[g0 + GPC]
        ncnt = gn1 - gn0
        assert ncnt <= TS * P, f"s2s overflow core {cidx}: {ncnt} > {TS*P}"
        s2s_idx = np.zeros((TS * P,), np.int32)
        s2s_idx[:ncnt] = np.arange(gn0, gn1, dtype=np.int32)
        s2s_mask = np.zeros((TS * P,), np.float32)
        s2s_mask[:ncnt] = 1.0
        s2s_brel = np.zeros((TS * P,), np.float32)
        s2s_brel[:ncnt] = (batch[gn0:gn1] - g0).astype(np.float32)

        m = dict(
            xT=np.ascontiguousarray(x[n0:n1].T).astype(BF),           # [ND, NPC]
            idx16_in=idx16, dcol_in=dcol_fl, mask_in=mask_fl, srow_in=seaT_fl,
            iota_in=np.broadcast_to(np.arange(P, dtype=np.float32),
                                    (P, P)).astype(BF).copy(),
            iotacol_in=np.arange(P, dtype=np.float32).reshape(P, 1).astype(BF),
            ones_in=np.ones((1, P), BF),
            s2s_idx_in=np.ascontiguousarray(s2s_idx.reshape(TS, P).T),
            s2s_mask_in=np.ascontiguousarray(s2s_mask.reshape(TS, P).T),
            s2s_brel_in=np.ascontiguousarray(s2s_brel.reshape(TS, P).T).astype(BF),
            s2s_brelr_in=s2s_brel.reshape(1, TS * P).astype(BF),
            gfT_in=np.ascontiguousarray(
                np.asarray(inp["global_features"], np.float32)[g0:g0 + GPC].T
            ).astype(BF),                                             # [GD, GPC]
        )
        for i, L in enumerate(layers):
            m[f"W{i}"] = L["W"].astype(BF)
            m[f"Mae{i}"] = L["M_ae"].astype(BF)
            m[f"bias{i}"] = np.broadcast_to(L["b"], (P, D)).copy()   # f32
        # s2s weights follow the c-major feature permutation: q_star rows and
        # per-gate output columns permute so <xn, h> stays consistent
        qperm = np.concatenate([PERM, PERM + D])
        gperm = np.concatenate([g * D + PERM for g in range(4)])
        WihT_p = np.asarray(inp["s2s_Wih"], np.float32).T[qperm][:, gperm]
        WhhT_p = np.asarray(inp["s2s_Whh"], np.float32).T[PERM][:, gperm]
        m["WihT"] = np.ascontiguousarray(WihT_p).astype(BF)          # [2D, 4D]
        m["WhhT"] = np.ascontiguousarray(WhhT_p).astype(BF)          # [D, 4D]
        m["s2s_bias"] = (np.asarray(inp["s2s_bih"], np.float32)
                         + np.asarray(inp["s2s_bhh"], np.float32)
                         )[gperm].reshape(1, -1).astype(BF)          # [1, 4D]
        p1_rows = np.concatenate([qperm, np.arange(2 * D, 2 * D + GD)])
        m["p1W"] = np.asarray(inp["p1_W"], np.float32)[p1_rows].astype(BF)
        m["p1b"] = np.asarray(inp["p1_b"], np.float32).reshape(1, -1).astype(BF)
        m["p2W"] = np.asarray(inp["p2_W"], np.float32).astype(BF)
        m["p2b"] = np.asarray(inp["p2_b"], np.float32).reshape(1, -1).astype(BF)
        m["p3W"] = np.asarray(inp["p3_W"], np.float32).astype(BF)
        m["p3b"] = np.asarray(inp["p3_b"], np.float32).reshape(1, -1).astype(BF)
        in_maps.append(m)
    return in_maps, pack


# ------------------------------------------------------------------
# device kernel builder
# ------------------------------------------------------------------

def build_kernel(cfg, pack, reps=1):
    N, NPC, W, GPC, TS = cfg.N, cfg.NPC, cfg.W, cfg.GPC, cfg.TS
    D, ED, GD = cfg.D, cfg.ED, cfg.GD
    TW = D + 8           # gather-table row width (lin 256 + a_src slot 8)
    HS = [8, 8, 8, 1]    # heads per layer
    DINS = [cfg.ND, D, D, D]
    NL = cfg.LAYERS
    TK = pack["TK"]

    nc = bacc.Bacc("TRN2", target_bir_lowering=False, debug=False,
                   num_devices=cfg.NC, num_swdge_queues=4)

    # ---------------- inputs ----------------
    xT = nc.dram_tensor("xT", [cfg.ND, NPC], BF16, kind="ExternalInput")
    idx16_in = nc.dram_tensor("idx16_in", [P, 8 * TK], I16, kind="ExternalInput")
    dcol_in = nc.dram_tensor("dcol_in", [P, TK], BF16, kind="ExternalInput")
    mask_in = nc.dram_tensor("mask_in", [P, TK], BF16, kind="ExternalInput")
    srow_in = nc.dram_tensor("srow_in", [ED, TK * P], BF16,
                             kind="ExternalInput")
    iota_in = nc.dram_tensor("iota_in", [P, P], BF16, kind="ExternalInput")
    iotacol_in = nc.dram_tensor("iotacol_in", [P, 1], BF16, kind="ExternalInput")
    ones_in = nc.dram_tensor("ones_in", [1, P], BF16, kind="ExternalInput")
    Wm, Mae, biasg = [], [], []
    for i in range(NL):
        Wm.append(nc.dram_tensor(f"W{i}", [DINS[i], D + 16], BF16,
                                 kind="ExternalInput"))
        Mae.append(nc.dram_tensor(f"Mae{i}", [ED, HS[i]], BF16, kind="ExternalInput"))
        biasg.append(nc.dram_tensor(f"bias{i}", [P, D], F32, kind="ExternalInput"))
    s2s_idx_in = nc.dram_tensor("s2s_idx_in", [P, TS], I32, kind="ExternalInput")
    s2s_mask_in = nc.dram_tensor("s2s_mask_in", [P, TS], F32, kind="ExternalInput")
    s2s_brel_in = nc.dram_tensor("s2s_brel_in", [P, TS], BF16, kind="ExternalInput")
    s2s_brelr_in = nc.dram_tensor("s2s_brelr_in", [1, TS * P], BF16,
                                  kind="ExternalInput")
    gfT_in = nc.dram_tensor("gfT_in", [GD, GPC], BF16, kind="ExternalInput")
    WihT = nc.dram_tensor("WihT", [2 * D, 4 * D], BF16, kind="ExternalInput")
    WhhT = nc.dram_tensor("WhhT", [D, 4 * D], BF16, kind="ExternalInput")
    s2s_bias = nc.dram_tensor("s2s_bias", [1, 4 * D], BF16, kind="ExternalInput")
    p1W = nc.dram_tensor("p1W", [2 * D + GD, D], BF16, kind="ExternalInput")
    p1b = nc.dram_tensor("p1b", [1, D], BF16, kind="ExternalInput")
    p2W = nc.dram_tensor("p2W", [D, D // 2], BF16, kind="ExternalInput")
    p2b = nc.dram_tensor("p2b", [1, D // 2], BF16, kind="ExternalInput")
    p3W = nc.dram_tensor("p3W", [D // 2, 5], BF16, kind="ExternalInput")
    p3b = nc.dram_tensor("p3b", [1, 5], BF16, kind="ExternalInput")
    out_t = nc.dram_tensor("out", [GPC, 5], F32, kind="ExternalOutput")
    if cfg.DBG:
        dbgh_t = nc.dram_tensor("dbgh", [N, D], BF16, kind="ExternalOutput")
        dbghs_t = [nc.dram_tensor(f"dbghs{li}", [P, W * D], BF16,
                                  kind="ExternalOutput") for li in range(NL - 1)]
        dbght_t = [nc.dram_tensor(f"dbght{li}", [P, W * 2 * P], BF16,
                                  kind="ExternalOutput") for li in range(NL - 1)]
        KMAX = pack["KMAX"]
        dbgM_t = nc.dram_tensor("dbgM", [P, KMAX * P], BF16,
                                kind="ExternalOutput")
        dbgMT_t = nc.dram_tensor("dbgMT", [P, KMAX * P], BF16,
                                 kind="ExternalOutput")
        dbgex_t = nc.dram_tensor("dbgex", [P, KMAX * 8], BF16,
                                 kind="ExternalOutput")
        dbgal_t = nc.dram_tensor("dbgal", [P, KMAX * 8], BF16,
                                 kind="ExternalOutput")
        dbglg_t = nc.dram_tensor("dbglg", [P, KMAX * TWP], BF16,
                                 kind="ExternalOutput")

    T = dict(locals())
    with tile.TileContext(nc) as tc:
        for rep in range(reps):
            build_body(nc, tc, cfg, pack, T, sfx=f"r{rep}" if reps > 1 else "")
    nc.compile()
    return nc


def build_body(nc, tc, cfg, pack, T, sfx=""):
    N, NPC, W, GPC, TS = cfg.N, cfg.NPC, cfg.W, cfg.GPC, cfg.TS
    D, ED, GD = cfg.D, cfg.ED, cfg.GD
    TW = D + 8
    HS = [8, 8, 8, 1]
    DINS = [cfg.ND, D, D, D]
    NL = cfg.LAYERS
    K_w, KOFF, TK, KMAX = pack["K_w"], pack["KOFF"], pack["TK"], pack["KMAX"]
    RG = [list(range(cfg.NC))]
    SLAB = 4             # phase-A windows per store slab

    import contextlib
    ctx = contextlib.ExitStack()
    with ctx:
        pers = ctx.enter_context(tc.tile_pool(name="pers", bufs=1))
        dram = ctx.enter_context(tc.tile_pool(name="dram", bufs=1, space="DRAM"))

        # ---- persistent constants ----
        iota_sb = pers.tile([P, P], BF16, tag="iota")
        nc.sync.dma_start(iota_sb[:], T["iota_in"][:])
        iotac_sb = pers.tile([P, 1], BF16, tag="iotac")
        nc.sync.dma_start(iotac_sb[:], T["iotacol_in"][:])
        ones_sb = pers.tile([1, P], BF16, tag="ones")
        nc.sync.dma_start(ones_sb[:], T["ones_in"][:])
        ident_sb = pers.tile([P, P], F32, tag="ident")
        nc.vector.tensor_tensor(out=ident_sb[:],
                                in0=iotac_sb[:].to_broadcast([P, P]),
                                in1=iota_sb[:], op=OP.is_equal)
        ident16_sb = pers.tile([P, P], BF16, tag="ident16")
        nc.vector.tensor_copy(ident16_sb[:], ident_sb[:])
        # iotaRep[p, n, k] = n — packed-innermost counterpart of iota for the
        # indicator build (keeps every operand's last dim stride-1 so the DVE
        # runs in its 2x/4x mode)
        KMAX = pack["KMAX"]
        iotarep_sb = pers.tile([P, P, KMAX], BF16, tag="iotarep")
        nc.vector.tensor_copy(
            iotarep_sb[:],
            iota_sb[:, :, None].to_broadcast([P, P, KMAX]))

        # edge metadata, resident for the whole kernel
        idx16_all = pers.tile([P, 8 * TK], I16, tag="idx16_all")
        nc.sync.dma_start(idx16_all[:], T["idx16_in"][:])
        dcol_all = pers.tile([P, TK], BF16, tag="dcol_all")
        nc.sync.dma_start(dcol_all[:], T["dcol_in"][:])
        mask_all = pers.tile([P, TK], BF16, tag="mask_all")
        nc.sync.dma_start(mask_all[:], T["mask_in"][:])

        # transposed features ping-pong, resident in SBUF:
        # window w chunk c lives at cols [w*2P + c*P : w*2P + c*P + P)
        hT_sb = [pers.tile([P, W * 2 * P], BF16, tag=f"hT{pp}",
                           name=f"hT{pp}{sfx}")
                 for pp in range(2)]
        # residual h (bf16), resident: window w at cols [w*D:(w+1)*D)
        h_sb = pers.tile([P, W * D], BF16, tag="h_sb")

        # DRAM scratch (rows padded to TWP for 256B-aligned dma_gather);
        # one Shared table tensor per window-range chunk per layer, rows
        # [core][p][w-in-chunk] — see host src2chunkrow
        NCH, WB = pack["NCH"], pack["WB"]
        lin_loc = [dram.tile([P, (WB[cb + 1] - WB[cb]) * TWP], BF16,
                             tag=f"lin_loc{cb}", name=f"lin_loc{cb}{sfx}")
                   for cb in range(NCH)]
        tables = [[dram.tile([cfg.NC * P * (WB[cb + 1] - WB[cb]), TWP], BF16,
                             tag=f"table{li}_{cb}",
                             name=f"table{li}_{cb}{sfx}",
                             addr_space="Shared") for cb in range(NCH)]
                  for li in range(NL)]
        hfin_local = dram.tile([NPC, D], BF16, tag="hfin_local")
        hfin_table = dram.tile([N, D], BF16, tag=f"hfin_table{sfx}",
                               addr_space="Shared")

        # ================= GAT layers =================
        with tc.tile_pool(name="lw", bufs=2) as lw, \
             tc.tile_pool(name="win", bufs=2) as win, \
             tc.tile_pool(name="psN", bufs=2, space="PSUM") as psN, \
             tc.tile_pool(name="psS", bufs=1, space="PSUM") as psS:
            for li in range(NL):
                H = HS[li]
                C = D // H
                din = DINS[li]
                nkc = (din + P - 1) // P   # contraction chunks for lin matmul

                # --- per-layer weights into SBUF ---
                DW = D + 16
                W_sb = lw.tile([P, nkc * DW], BF16, tag="W_sb")
                for c2 in range(nkc):
                    r0, r1 = c2 * P, min((c2 + 1) * P, din)
                    nc.sync.dma_start(W_sb[: r1 - r0, c2 * DW:(c2 + 1) * DW],
                                      T["Wm"][li][r0:r1, :])
                if pack["use_bias"]:
                    bias_b = lw.tile([P, D], F32, tag="bias_b")
                    nc.sync.dma_start(bias_b[:], T["biasg"][li][:])
                mae_sb = lw.tile([ED, 8], BF16, tag="mae_sb")
                nc.sync.dma_start(mae_sb[:, :H], T["Mae"][li][:])
                adst_all = lw.tile([P, W * 8], BF16, tag="adst_all")
                nc.vector.memset(adst_all[:], 0.0)

                hT_prev = hT_sb[li % 2]
                hT_next = hT_sb[(li + 1) % 2]

                # ---------- phase A: dense lin + a_src/a_dst ----------
                # slabs iterate within AllGather chunks so each chunk's
                # staging buffer is written contiguously and its collective
                # can fire as soon as the chunk's last slab lands
                phaseA_slabs = [(cb, s0, min(s0 + SLAB, WB[cb + 1]))
                                for cb in range(NCH)
                                for s0 in range(WB[cb], WB[cb + 1], SLAB)]
                for cb_, s0, s1 in phaseA_slabs:
                    lin16 = win.tile([P, SLAB, TWP], BF16, tag="lin16")
                    if li == 0:
                        rows0 = min(s1 * P, NPC) - s0 * P
                        xTw = win.tile([cfg.ND, SLAB * P], BF16, tag="xTw")
                        nc.sync.dma_start(xTw[:, :rows0],
                                          T["xT"][:, s0 * P:s0 * P + rows0])
                    for w in range(s0, s1):
                        n0 = w * P
                        cnt = min(P, NPC - n0)
                        j = w - s0
                        lin_ps = psS.tile([P, D + 16], F32, tag="scr_ps")
                        if li == 0:
                            nc.tensor.matmul(lin_ps[:cnt],
                                             lhsT=xTw[:, j * P:j * P + cnt],
                                             rhs=W_sb[:din, 0:DW],
                                             start=True, stop=True)
                        else:
                            for c2 in range(nkc):
                                nc.tensor.matmul(
                                    lin_ps[:cnt],
                                    lhsT=hT_prev[:, w * 2 * P + c2 * P:
                                                 w * 2 * P + c2 * P + cnt],
                                    rhs=W_sb[:, c2 * DW:(c2 + 1) * DW],
                                    start=(c2 == 0), stop=(c2 == nkc - 1))
                        # cols 0:D = lin (c-major), D:D+8 = a_src, D+8: = a_dst
                        nc.vector.tensor_copy(lin16[:cnt, j, 0:D + 8],
                                              lin_ps[:cnt, 0:D + 8])
                        nc.vector.tensor_copy(adst_all[:cnt, w * 8:w * 8 + 8],
                                              lin_ps[:cnt, D + 8:D + 16])
                    nj = s1 - s0
                    nc.sync.dma_start(
                        lin_loc[cb_][:, (s0 - WB[cb_]) * TWP:
                                     (s1 - WB[cb_]) * TWP],
                        lin16[:, :nj, :].rearrange("p j c -> p (j c)"))

                # ---------- AllGather the transformed-feature table ----------
                # chunked by window range so each chunk fires as soon as its
                # phase-A slabs land, overlapping the collective with the
                # remaining phase A / previous phase B work
                for cb in range(NCH):
                    w0, w1 = WB[cb], WB[cb + 1]
                    if cfg.FAKE_AG:
                        nc.sync.dma_start(
                            tables[li][cb][0:P * (w1 - w0), :].rearrange(
                                "(p w) c -> p (w c)", p=P),
                            lin_loc[cb][:])
                    else:
                        nc.gpsimd.collective_compute(
                            "AllGather", OP.bypass, replica_groups=RG,
                            ins=[lin_loc[cb][:]],
                            outs=[tables[li][cb][:]])

                # ---------- phase B: per-window edge aggregation ----------
                qrot = [0]
                for w in range(W):
                    n0 = w * P
                    cnt = min(P, NPC - n0)
                    K = K_w[w]
                    o0 = int(KOFF[w])
                    seaT_sb = win.tile([ED, KMAX * P], BF16, tag="seaT_sb")
                    nc.sync.dma_start(seaT_sb[:, :K * P],
                                      T["srow_in"][:, o0 * P:(o0 + K) * P])

                    lin_g = win.tile([P, KMAX, TWP], BF16, tag="lin_g")
                    tbase = 0
                    for cb in range(NCH):
                        tcnt = pack["TG_w"][w][cb]
                        for g0 in range(0, tcnt, 7):
                            g1 = min(g0 + 7, tcnt)
                            t0, t1 = tbase + g0, tbase + g1
                            nc.gpsimd.dma_gather(
                                out_ap=lin_g[:, t0:t1, :],
                                in_ap=tables[li][cb][:],
                                idxs_ap=idx16_all[:, 8 * (o0 + t0):
                                                  8 * (o0 + t1)],
                                num_idxs=(t1 - t0) * P,
                                num_idxs_reg=(t1 - t0) * P,
                                elem_size=TWP, queue_num=qrot[0] % 4)
                            qrot[0] += 1
                        tbase += tcnt

                    # indicator M [edge_p, node, k] (k innermost => DVE 2x);
                    # MT = M^T via PE transpose + scalar-engine PSUM copy
                    M_sb = win.tile([P, P, KMAX], BF16, tag="M_sb")
                    nc.vector.tensor_tensor(
                        out=M_sb[:, :, :K],
                        in0=dcol_all[:, None, o0:o0 + K].to_broadcast(
                            [P, P, K]),
                        in1=iotarep_sb[:, :, :K],
                        op=OP.is_equal)
                    MT_sb = win.tile([P, KMAX * P], BF16, tag="MT_sb")
                    trM_ps = psS.tile([P, KMAX * P], BF16, tag="trM_ps")
                    for k in range(K):
                        nc.tensor.transpose(trM_ps[:, k * P:(k + 1) * P],
                                            M_sb[:, :, k], ident16_sb[:])
                    nc.scalar.activation(MT_sb[:, :K * P], trM_ps[:, :K * P],
                                         AF.Copy)

                    # alpha = lrelu(a_src + a_dst + a_edge); ex = exp * mask
                    # (al shares the PSUM bank with nu at columns 264+)
                    acc_ps = psN.tile([P, D + 8 + KMAX * 8], F32, tag="acc_ps")
                    AL0 = D + 8
                    for k in range(K):
                        nc.tensor.matmul(acc_ps[:, AL0 + k * 8:AL0 + k * 8 + H],
                                         lhsT=MT_sb[:, k * P:(k + 1) * P],
                                         rhs=adst_all[:, w * 8:w * 8 + H],
                                         start=True, stop=False)
                        nc.tensor.matmul(acc_ps[:, AL0 + k * 8:AL0 + k * 8 + H],
                                         lhsT=seaT_sb[:, k * P:(k + 1) * P],
                                         rhs=mae_sb[:, :H],
                                         start=False, stop=True)
                    al_sb = win.tile([P, KMAX * 8], BF16, tag="al_sb")
                    nc.vector.tensor_tensor(
                        out=al_sb[:, :K * 8].rearrange(
                            "p (k h) -> p k h", k=K)[:, :, :H],
                        in0=acc_ps[:, AL0:AL0 + K * 8].rearrange(
                            "p (k h) -> p k h", k=K)[:, :, :H],
                        in1=lin_g[:, :K, D:D + H], op=OP.add)
                    # leaky_relu(x, 0.2) = max(0.2*x, x)
                    lr_sb = win.tile([P, KMAX * 8], BF16, tag="lr_sb")
                    nc.scalar.activation(lr_sb[:, :K * 8], al_sb[:, :K * 8],
                                         AF.Lrelu, alpha=0.2)
                    ex_sb = win.tile([P, KMAX * 8], BF16, tag="ex_sb")
                    nc.scalar.activation(ex_sb[:, :K * 8], lr_sb[:, :K * 8],
                                         AF.Exp)
                    # mask pad slots; for H=1 also duplicate the per-edge
                    # weight across the 8 lanes so the wfex multiply can use
                    # the packed-h fast path
                    nc.vector.tensor_tensor(
                        out=ex_sb[:, :K * 8].rearrange("p (k h) -> p k h", k=K),
                        in0=(ex_sb[:, :K * 8].rearrange("p (k h) -> p k h",
                                                        k=K)
                             if H > 1 else
                             ex_sb[:, :K * 8].rearrange("p (k h) -> p k h",
                                                        k=K)[:, :, 0:1
                                 ].to_broadcast([P, K, 8])),
                        in1=mask_all[:, o0:o0 + K][:, :, None].to_broadcast(
                            [P, K, 8]),
                        op=OP.mult)

                    if cfg.DBG and li == 0 and w == 0:
                        nc.sync.dma_start(T["dbgM_t"][:, :K * P],
                                          M_sb[:, :K, :].rearrange(
                                              "p k n -> p (k n)"))
                        nc.sync.dma_start(T["dbgMT_t"][:, :K * P],
                                          MT_sb[:, :K * P])
                        nc.sync.dma_start(T["dbgex_t"][:, :K * 8],
                                          ex_sb[:, :K * 8])
                        nc.sync.dma_start(T["dbgal_t"][:, :K * 8],
                                          al_sb[:, :K * 8])
                        nc.sync.dma_start(T["dbglg_t"][:, :K * TWP],
                                          lin_g[:, :K, :].rearrange(
                                              "p k c -> p (k c)"))

                    # weighted features + segment sums (low half of acc_ps)
                    for b0 in range(0, K, 4):
                        b1 = min(b0 + 4, K)
                        nb = b1 - b0
                        wfex = win.tile([P, 4, D + 8], BF16, tag="wfex")
                        # c-major feature layout: the 8-lane dim is the packed
                        # innermost dim for all three operands (for H=1 the
                        # lanes hold 8 copies of the per-edge weight)
                        nc.vector.tensor_tensor(
                            out=wfex[:, :nb, 0:D].rearrange(
                                "p k (c h) -> p k c h", h=8),
                            in0=lin_g[:, b0:b1, 0:D].rearrange(
                                "p k (c h) -> p k c h", h=8),
                            in1=ex_sb[:, b0 * 8:b1 * 8].rearrange(
                                "p (k h) -> p k h", k=nb)[:, :, None, :
                                ].to_broadcast([P, nb, D // 8, 8]),
                            op=OP.mult)
                        nc.vector.tensor_copy(
                            wfex[:, :nb, D:D + 8],
                            ex_sb[:, b0 * 8:b1 * 8].rearrange(
                                "p (k h) -> p k h", k=nb))
                        for kk in range(nb):
                            k = b0 + kk
                            nc.tensor.matmul(
                                acc_ps[:, 0:D + H],
                                lhsT=M_sb[:, :, k],
                                rhs=wfex[:, kk, 0:D + H],
                                start=(k == 0), stop=(k == K - 1))

                    # normalize, bias, ELU, residual (bf16 epilogue); den > 0
                    # is guaranteed by the self-loop edge, so no epsilon
                    rec = win.tile([P, 8], F32, tag="rec")
                    nc.vector.reciprocal(rec[:cnt, :H], acc_ps[:cnt, D:D + H])
                    outw = win.tile([P, D], BF16, tag="outw")
                    nc.vector.tensor_tensor(
                        out=outw[:cnt].rearrange("p (c h) -> p c h", h=H),
                        in0=acc_ps[:cnt, 0:D].rearrange("p (c h) -> p c h",
                                                        h=H),
                        in1=rec[:cnt, None, :H].to_broadcast(
                            [cnt, D // H, H]),
                        op=OP.mult)
                    if pack["use_bias"]:
                        nc.vector.tensor_tensor(out=outw[:cnt], in0=outw[:cnt],
                                                in1=bias_b[:cnt], op=OP.add)
                    # ELU = relu(x) + exp(min(x,0)) - 1
                    tmin = win.tile([P, D], BF16, tag="tmin")
                    nc.vector.tensor_scalar_min(tmin[:cnt], outw[:cnt], 0.0)
                    nc.scalar.activation(tmin[:cnt], tmin[:cnt], AF.Exp)
                    nc.vector.tensor_scalar_max(outw[:cnt], outw[:cnt], 0.0)
                    hn = win.tile([P, D], BF16, tag="hn")
                    if cnt < P:
                        nc.vector.memset(hn[:], 0.0)
                    nc.vector.tensor_tensor(out=hn[:cnt], in0=tmin[:cnt],
                                            in1=outw[:cnt], op=OP.add)
                    nc.vector.tensor_scalar_add(hn[:cnt], hn[:cnt], -1.0)
                    if li > 0:
                        nc.vector.tensor_tensor(out=hn[:cnt], in0=hn[:cnt],
                                                in1=h_sb[:cnt, w * D:w * D + D],
                                                op=OP.add)
                    if li < NL - 1:
                        nc.vector.tensor_copy(h_sb[:, w * D:(w + 1) * D], hn[:])
                        tr16_ps = psS.tile([P, D], BF16, tag="tr16_ps")
                        for c2 in range(2):
                            nc.tensor.transpose(tr16_ps[:, c2 * P:(c2 + 1) * P],
                                                hn[:, c2 * P:(c2 + 1) * P],
                                                ident16_sb[:])
                            nc.vector.tensor_copy(
                                hT_next[:, w * 2 * P + c2 * P:
                                        w * 2 * P + c2 * P + cnt],
                                tr16_ps[:, c2 * P:c2 * P + cnt])
                    else:
                        nc.sync.dma_start(hfin_local[n0:n0 + cnt], hn[:cnt])

                if cfg.DBG and li < NL - 1:
                    nc.sync.dma_start(T["dbghs_t"][li][:], h_sb[:])
                    nc.sync.dma_start(T["dbght_t"][li][:], hT_next[:])

            # final AllGather of node features for set2set
            if cfg.FAKE_AG:
                nc.sync.dma_start(hfin_table[0:NPC, :], hfin_local[:])
            else:
                nc.gpsimd.collective_compute(
                    "AllGather", OP.bypass, replica_groups=RG,
                    ins=[hfin_local[:]], outs=[hfin_table[:]])
            if cfg.DBG:
                nc.sync.dma_start(T["dbgh_t"][:], hfin_table[:])

        # ================= Set2Set + MLP head =================
        build_s2s(nc, tc, cfg, T, pers, dram, hfin_table,
                  iota_sb, iotac_sb, ones_sb, ident_sb, sfx=sfx)


def build_s2s(nc, tc, cfg, T, pers, dram, hfin_table,
              iota_sb, iotac_sb, ones_sb, ident_sb, sfx=""):
    N, NPC, GPC, TS = cfg.N, cfg.NPC, cfg.GPC, cfg.TS
    D, GD = cfg.D, cfg.GD
    GG = GPC
    STEPS = cfg.S2S_STEPS

    with tc.tile_pool(name="s2s", bufs=1) as sp, \
         tc.tile_pool(name="ps2", bufs=1, space="PSUM") as ps2:
        # gather this core's node features (padded to TS*128)
        s2s_idx = sp.tile([P, TS], I32, tag="s2s_idx")
        nc.sync.dma_start(s2s_idx[:], T["s2s_idx_in"][:])
        xn = sp.tile([P, TS, D], BF16, tag="xn")
        for t in range(TS):
            nc.gpsimd.indirect_dma_start(
                out=xn[:, t, :], out_offset=None, in_=hfin_table[:],
                in_offset=bass.IndirectOffsetOnAxis(
                    ap=s2s_idx[:, t:t + 1], axis=0))
        maskc = sp.tile([P, TS], F32, tag="maskc")
        nc.sync.dma_start(maskc[:], T["s2s_mask_in"][:])
        brelc = sp.tile([P, TS], BF16, tag="brelc")
        nc.sync.dma_start(brelc[:], T["s2s_brel_in"][:])
        brelr = sp.tile([1, TS * P], BF16, tag="brelr")
        nc.sync.dma_start(brelr[:], T["s2s_brelr_in"][:])

        # indicator matrices per node tile (once for all steps)
        Mb = sp.tile([P, TS * GG], BF16, tag="Mb")       # node_p x graph_f
        nc.vector.tensor_tensor(
            out=Mb[:].rearrange("p (t g) -> p t g", t=TS),
            in0=brelc[:, :, None].to_broadcast([P, TS, GG]),
            in1=iota_sb[:, None, 0:GG].to_broadcast([P, TS, GG]),
            op=OP.is_equal)
        MbT = sp.tile([GG, TS * P], BF16, tag="MbT")     # graph_p x node_f
        for t in range(TS):
            bc_ps = ps2.tile([GG, P], F32, tag="psX")
            nc.tensor.matmul(bc_ps[:], lhsT=ones_sb[:, 0:GG],
                             rhs=brelr[:, t * P:(t + 1) * P], start=True,
                             stop=True)
            nc.vector.tensor_tensor(
                out=MbT[:, t * P:(t + 1) * P],
                in0=iotac_sb[:GG].to_broadcast([GG, P]),
                in1=bc_ps[:], op=OP.is_equal)

        # s2s weights: WihT [2D,4D] in 4 row-chunks, WhhT [D,4D] in 2 chunks
        wih = sp.tile([P, 4 * 4 * D], BF16, tag="wih")
        for c2 in range(4):
            nc.sync.dma_start(wih[:, c2 * 4 * D:(c2 + 1) * 4 * D],
                              T["WihT"][c2 * P:(c2 + 1) * P, :])
        whh = sp.tile([P, 2 * 4 * D], BF16, tag="whh")
        for c2 in range(2):
            nc.sync.dma_start(whh[:, c2 * 4 * D:(c2 + 1) * 4 * D],
                              T["WhhT"][c2 * P:(c2 + 1) * P, :])
        s2sb = sp.tile([1, 4 * D], BF16, tag="s2sb")
        nc.sync.dma_start(s2sb[:], T["s2s_bias"][:])

        # LSTM / attention state: q*^T chunks (h part then r part), h^T, c
        qT = [sp.tile([P, GG], BF16, tag=f"qT{c2}", name=f"qT{c2}{sfx}")
              for c2 in range(4)]
        c_st = sp.tile([GG, D], F32, tag="c_st")
        for t_ in qT:
            nc.vector.memset(t_[:], 0.0)
        nc.vector.memset(c_st[:], 0.0)

        gact = [AF.Sigmoid, AF.Sigmoid, AF.Tanh, AF.Sigmoid]  # i, f, g, o
        for step in range(STEPS):
            gs = []
            for g in range(4):
                g_ps = ps2.tile([GG, D], F32, tag="psY")
                nc.tensor.matmul(g_ps[:], lhsT=ones_sb[:, 0:GG],
                                 rhs=s2sb[:, g * D:(g + 1) * D],
                                 start=True, stop=False)
                for c2 in range(4):
                    nc.tensor.matmul(
                        g_ps[:], lhsT=qT[c2][:],
                        rhs=wih[:, c2 * 4 * D + g * D: c2 * 4 * D + (g + 1) * D],
                        start=False, stop=False)
                for c2 in range(2):
                    # h part of q_star doubles as the LSTM h for Whh
                    nc.tensor.matmul(
                        g_ps[:], lhsT=qT[c2][:],
                        rhs=whh[:, c2 * 4 * D + g * D: c2 * 4 * D + (g + 1) * D],
                        start=False, stop=(c2 == 1))
                g_sb = sp.tile([GG, D], F32, tag=f"g_sb{g}")
                nc.scalar.activation(g_sb[:], g_ps[:], gact[g])
                gs.append(g_sb)
            # c = f*c + i*tanh(g);  h = o*tanh(c)
            t1 = sp.tile([GG, D], F32, tag="t1")
            nc.vector.tensor_tensor(out=t1[:], in0=gs[0][:], in1=gs[2][:],
                                    op=OP.mult)
            nc.vector.tensor_tensor(out=c_st[:], in0=gs[1][:], in1=c_st[:],
                                    op=OP.mult)
            nc.vector.tensor_tensor(out=c_st[:], in0=c_st[:], in1=t1[:],
                                    op=OP.add)
            tc_sb = sp.tile([GG, D], F32, tag="tc_sb")
            nc.scalar.activation(tc_sb[:], c_st[:], AF.Tanh)
            h_l = sp.tile([GG, D], F32, tag="h_l")
            nc.vector.tensor_tensor(out=h_l[:], in0=gs[3][:], in1=tc_sb[:],
                                    op=OP.mult)
            h16 = sp.tile([GG, D], BF16, tag="h16")
            nc.vector.tensor_copy(h16[:], h_l[:])

            # attention over nodes: e = <xn, h[batch]>, softmax per graph
            e_all = sp.tile([P, TS], F32, tag="e_all")
            escr = sp.tile([P, D], F32, tag="escr")
            for t in range(TS):
                he_ps = ps2.tile([P, D], F32, tag="psH")
                nc.tensor.matmul(he_ps[:], lhsT=MbT[:, t * P:(t + 1) * P],
                                 rhs=h16[:], start=True, stop=True)
                nc.vector.tensor_tensor(out=escr[:], in0=xn[:, t, :],
                                        in1=he_ps[:], op=OP.mult)
                nc.vector.reduce_sum(out=e_all[:, t:t + 1], in_=escr[:],
                                     axis=mybir.AxisListType.X)
            nc.scalar.activation(e_all[:], e_all[:], AF.Exp)
            nc.vector.tensor_tensor(out=e_all[:], in0=e_all[:], in1=maskc[:],
                                    op=OP.mult)
            e16 = sp.tile([P, TS], BF16, tag="e16")
            nc.vector.tensor_copy(e16[:], e_all[:])
            r_ps = ps2.tile([GG, D + 1], F32, tag="psR")
            for t in range(TS):
                wxex = sp.tile([P, D + 1], BF16, tag="wxex")
                nc.vector.tensor_tensor(
                    out=wxex[:, 0:D], in0=xn[:, t, :],
                    in1=e16[:, t:t + 1].to_broadcast([P, D]), op=OP.mult)
                nc.vector.tensor_copy(wxex[:, D:D + 1], e16[:, t:t + 1])
                nc.tensor.matmul(r_ps[:], lhsT=Mb[:, t * GG:(t + 1) * GG],
                                 rhs=wxex[:], start=(t == 0), stop=(t == TS - 1))
            den = sp.tile([GG, 1], F32, tag="s2s_den")
            nc.vector.tensor_scalar_add(den[:], r_ps[:, D:D + 1], 1e-16)
            rec = sp.tile([GG, 1], F32, tag="s2s_rec")
            nc.vector.reciprocal(rec[:], den[:])
            r_sb = sp.tile([GG, D], F32, tag="r_sb")
            nc.vector.tensor_tensor(out=r_sb[:], in0=r_ps[:, 0:D],
                                    in1=rec[:].to_broadcast([GG, D]),
                                    op=OP.mult)
            # q_star^T = [h^T | r^T] for next step / head
            for c2 in range(2):
                tr_ps = ps2.tile([P, GG], F32, tag="psX")
                nc.tensor.transpose(tr_ps[:], h_l[:, c2 * P:(c2 + 1) * P],
                                    ident_sb[:GG, :GG])
                nc.vector.tensor_copy(qT[c2][:], tr_ps[:])
                tr_ps2 = ps2.tile([P, GG], F32, tag="psX")
                nc.tensor.transpose(tr_ps2[:], r_sb[:, c2 * P:(c2 + 1) * P],
                                    ident_sb[:GG, :GG])
                nc.vector.tensor_copy(qT[2 + c2][:], tr_ps2[:])

        # ---------------- MLP head ----------------
        gfT_sb = sp.tile([GD, GG], BF16, tag="gfT_sb")
        nc.sync.dma_start(gfT_sb[:], T["gfT_in"][:])
        p1w_sb = sp.tile([P, 4 * D], BF16, tag="p1w_sb")
        for c2 in range(4):
            nc.sync.dma_start(p1w_sb[:, c2 * D:(c2 + 1) * D],
                              T["p1W"][c2 * P:(c2 + 1) * P, :])
        p1wg_sb = sp.tile([GD, D], BF16, tag="p1wg_sb")
        nc.sync.dma_start(p1wg_sb[:], T["p1W"][4 * P:4 * P + GD, :])
        p1b_sb = sp.tile([1, D], BF16, tag="p1b_sb")
        nc.sync.dma_start(p1b_sb[:], T["p1b"][:])
        z1_ps = ps2.tile([GG, D], F32, tag="psY")
        nc.tensor.matmul(z1_ps[:], lhsT=ones_sb[:, 0:GG], rhs=p1b_sb[:],
                         start=True, stop=False)
        for c2 in range(4):
            nc.tensor.matmul(z1_ps[:], lhsT=qT[c2][:],
                             rhs=p1w_sb[:, c2 * D:(c2 + 1) * D],
                             start=False, stop=False)
        nc.tensor.matmul(z1_ps[:], lhsT=gfT_sb[:], rhs=p1wg_sb[:],
                         start=False, stop=True)
        z1 = sp.tile([GG, D], F32, tag="z1")
        nc.scalar.activation(z1[:], z1_ps[:], AF.Relu)

        p2w_sb = sp.tile([P, 2 * (D // 2)], BF16, tag="p2w_sb")
        for c2 in range(2):
            nc.sync.dma_start(p2w_sb[:, c2 * (D // 2):(c2 + 1) * (D // 2)],
                              T["p2W"][c2 * P:(c2 + 1) * P, :])
        p2b_sb = sp.tile([1, D // 2], BF16, tag="p2b_sb")
        nc.sync.dma_start(p2b_sb[:], T["p2b"][:])
        z2_ps = ps2.tile([GG, D // 2], F32, tag="psY")
        nc.tensor.matmul(z2_ps[:], lhsT=ones_sb[:, 0:GG], rhs=p2b_sb[:],
                         start=True, stop=False)
        for c2 in range(2):
            z1T_ps = ps2.tile([P, GG], F32, tag="psX")
            nc.tensor.transpose(z1T_ps[:], z1[:, c2 * P:(c2 + 1) * P],
                                ident_sb[:GG, :GG])
            z1T = sp.tile([P, GG], BF16, tag="z1T")
            nc.vector.tensor_copy(z1T[:], z1T_ps[:])
            nc.tensor.matmul(z2_ps[:], lhsT=z1T[:],
                             rhs=p2w_sb[:, c2 * (D // 2):(c2 + 1) * (D // 2)],
                             start=False, stop=(c2 == 1))
        z2 = sp.tile([GG, D // 2], F32, tag="z2")
        nc.scalar.activation(z2[:], z2_ps[:], AF.Relu)

        p3w_sb = sp.tile([D // 2, 5], BF16, tag="p3w_sb")
        nc.sync.dma_start(p3w_sb[:], T["p3W"][:])
        p3b_sb = sp.tile([1, 5], BF16, tag="p3b_sb")
        nc.sync.dma_start(p3b_sb[:], T["p3b"][:])
        z2T_ps = ps2.tile([P, GG], F32, tag="psX")
        nc.tensor.transpose(z2T_ps[:], z2[:], ident_sb[:GG, :GG])
        z2T = sp.tile([P, GG], BF16, tag="z2T")
        nc.vector.tensor_copy(z2T[:], z2T_ps[:])
        o_ps = ps2.tile([GG, 5], F32, tag="psY")
        nc.tensor.matmul(o_ps[:], lhsT=ones_sb[:, 0:GG], rhs=p3b_sb[:],
                         start=True, stop=False)
        nc.tensor.matmul(o_ps[:], lhsT=z2T[:], rhs=p3w_sb[:],
                         start=False, stop=True)
        o_sb = sp.tile([GG, 5], F32, tag="o_sb")
        nc.vector.tensor_copy(o_sb[:], o_ps[:])
        nc.sync.dma_start(T["out_t"][:], o_sb[:cfg.GPC])


def run_config(inputs, cfg):
    in_maps, pack = host_prep(inputs, cfg)
    nc = build_kernel(cfg, pack)
    res = run_bass_kernel_spmd(nc, in_maps, core_ids=list(range(cfg.NC)))
    out = np.concatenate([res.results[c]["out"] for c in range(cfg.NC)], axis=0)
    return out.astype(np.float32)


def kernel(**inputs):
    return run_config(inputs, CFG.derive())
